# revision 49
# baseline (speedup 1.0000x reference)
"""AttentionBlock (GroupNorm + single-head 1x1-conv attention + residual) on 8 trn2 cores.

Sharding: 8 cores = (batch b in 0..3) x (query-half h in 0..1). Each core computes
the full attention rows for its 2048 query positions of its sample.

Per-core pipeline (all matmuls in float32r: full PE rate for moving-dim >= 256,
near-fp32 precision in practice):
  1. x loads in pieces; GroupNorm(1 group) stats via bn_stats overlapped with the
     DMA; the cross-partition combine runs on GpSimd (partition_all_reduce) so
     the PE instruction stream is never blocked by statistics.
  2. qkv weights are scaled by gamma only (known immediately), so all QKV
     matmuls start as soon as x pieces land. The scalar rstd and the effective
     q-bias are folded into Q alone: Q~ = rstd^2*q_raw + rstd*bq. Then
     logits = Q~^T K_raw reproduces rstd^2*q.k + u[j]; the k-bias and the
     constant terms cancel inside softmax. K and V therefore get plain
     (stats-free) PSUM->SBUF rounding copies. The V-path rstd is folded into
     the proj weights, and the V bias exactly into the proj bias (softmax rows
     sum to 1): out = (rstd*Wp) @ ao_raw + (Wp @ bv + pb).
  3. Q[c, i] (own half), K[c, j] (full), V^T[j, c] (full, produced transposed
     directly by swapping matmul operands).
  4. Streamed attention over 32 j-blocks x 1024-wide i-chunks: logits^T[jb, i]
     = K_blk^T Q~ into a 2-bank PSUM tile, one fused scale+exp on ScalarE per
     block, denominator partials alternating VectorE/GpSimd (no port contention
     at 1x), PV accumulated over j-blocks in PSUM. Softmax max-subtraction is
     skipped: logits are bounded (|l| <= 16) so exp is safe in fp32 and the
     result is mathematically identical.
  5. Denominator replicated across partitions with a ones-matmul, reciprocal,
     applied to PV. proj + bias + residual per chunk, overlapping the next
     chunk's attention.
"""

import os

import numpy as np

import concourse.bass as bass
import concourse.bass_isa as bass_isa
import concourse.mybir as mybir
import concourse.tile as tile
from concourse import bacc
from concourse.bass_utils import run_bass_kernel_spmd

# perf-experiment knobs (only read at build time; defaults = production)
_KNOB_NCH = int(os.environ.get("K_NCH", "2"))  # attention i-chunks emitted
_KNOB_MMBUFS = int(os.environ.get("K_MMBUFS", "2"))
_KNOB_EXBUFS = int(os.environ.get("K_EXBUFS", "3"))
_KNOB_STAGE = int(os.environ.get("K_STAGE", "9"))

B, C, HW = 4, 256, 4096
CHW0 = 1024  # attention i-chunk width == qk psum tile width (2 banks)
P, CO = 128, 2
NQ = HW // 2  # queries per core
NCORES = 8
SCALE = float(C) ** -0.5
EPS = 1e-5
F32, F32R = mybir.dt.float32, mybir.dt.float32r
Act = mybir.ActivationFunctionType
Alu = mybir.AluOpType

_prog = None


def _co_view(ap):
    """[C, N] dram AP -> [128, 2, N] (partition cp, free (co, n)); c = co*128+cp."""
    return ap.rearrange("(co cp) n -> cp co n", cp=P)


def _build():
    nc = bacc.Bacc(None, target_bir_lowering=False)
    _build_body(nc)
    nc.compile()
    return nc


def _build_body(nc):
    xr = nc.dram_tensor("xr", [C, HW], F32, kind="ExternalInput")
    xq = nc.dram_tensor("xq", [C, NQ], F32, kind="ExternalInput")
    wqkvT = nc.dram_tensor("wqkvT", [C, 3 * C], F32, kind="ExternalInput")
    wprojT = nc.dram_tensor("wprojT", [C, C], F32, kind="ExternalInput")
    qkvb = nc.dram_tensor("qkvb", [1, 3 * C], F32, kind="ExternalInput")
    projb = nc.dram_tensor("projb", [1, C], F32, kind="ExternalInput")
    gamma = nc.dram_tensor("gamma", [1, C], F32, kind="ExternalInput")
    beta = nc.dram_tensor("beta", [1, C], F32, kind="ExternalInput")
    y = nc.dram_tensor("y", [C, NQ], F32, kind="ExternalOutput")

    with tile.TileContext(nc) as tc:
        with (
            tc.tile_pool(name="big", bufs=1) as big,
            tc.tile_pool(name="wts", bufs=1) as wts,
            tc.tile_pool(name="small", bufs=1) as small,
            tc.tile_pool(name="expp", bufs=_KNOB_EXBUFS) as expp,
            tc.tile_pool(name="dap", bufs=1) as dap,
            tc.tile_pool(name="outp", bufs=2) as outp,
            tc.tile_pool(name="mm", bufs=_KNOB_MMBUFS, space="PSUM") as mmp,
            tc.tile_pool(name="pv", bufs=1, space="PSUM") as pvp,
            tc.tile_pool(name="dram", bufs=1, space="DRAM") as dramp,
        ):
            # ---- x load as 4 separate piece tiles (whole-tile dep tracking:
            # separate tiles let bn/K/VT start as each piece lands) ----
            NXP = 4
            XPW = HW // NXP  # 1024
            xps = []
            stt = small.tile([P, 16, 6], F32)
            for p_ in range(NXP):
                nsl = slice(p_ * XPW, (p_ + 1) * XPW)
                xp = big.tile([P, CO, XPW], F32R, tag=f"xp{p_}", name=f"xp{p_}")
                xps.append(xp)
                nc.sync.dma_start(xp[:], _co_view(xr[:])[:, :, nsl].bitcast(F32R))
                for co in range(CO):
                    for s2 in range(XPW // 512):
                        nc.vector.bn_stats(
                            out=stt[:, p_ * 4 + co * 2 + s2, :],
                            in_=xp[:, co, s2 * 512 : (s2 + 1) * 512].bitcast(F32),
                        )
            # xq rides the same (sync) HWDGE ring so it starts after the
            # stats-critical xr pieces have drained; split in half so chunk 0's
            # Q matmuls start a transfer earlier
            xqh = []
            for hh in range(2):
                t = big.tile([P, CO, NQ // 2], F32R, tag=f"xqh{hh}", name=f"xqh{hh}")
                xqh.append(t)
                nc.sync.dma_start(
                    t[:],
                    _co_view(xq[:])[:, :, hh * (NQ // 2) : (hh + 1) * (NQ // 2)]
                    .bitcast(F32R),
                )

            # ---- weights (gamma-only scale: no stats dependency) ----
            # wq first on the scalar HWDGE ring: it gates the first K matmuls
            wq = wts.tile([P, CO, 3 * C], F32R)
            nc.scalar.dma_start(wq[:], _co_view(wqkvT[:]).bitcast(F32R))
            gam = small.tile([P, CO], F32)
            nc.scalar.dma_start(gam[:], gamma[:][0].rearrange("(co cp) -> cp co", cp=P))
            bet = small.tile([P, CO], F32)
            nc.scalar.dma_start(bet[:], beta[:][0].rearrange("(co cp) -> cp co", cp=P))
            wqs = wts.tile([P, CO, 3 * C], F32R)
            for co in range(CO):
                nc.vector.tensor_scalar_mul(
                    out=wqs[:, co, :],
                    in0=wq[:, co, :].bitcast(F32),
                    scalar1=gam[:, co : co + 1],
                )
            wpj = wts.tile([P, CO, C], F32R)
            nc.scalar.dma_start(wpj[:], _co_view(wprojT[:]).bitcast(F32R))
            qb_sb = small.tile([1, 3 * C], F32)
            nc.scalar.dma_start(qb_sb[:1, :], qkvb[:])
            pb_sb = small.tile([1, C], F32)
            nc.scalar.dma_start(pb_sb[:1, :], projb[:])

            # ---- groupnorm stats (DVE + GpSimd only; PE never stalls) ----
            mv = small.tile([P, 2], F32)
            nc.vector.bn_aggr(out=mv[:], in_=stt[:])
            st2 = small.tile([P, 2], F32)
            nc.vector.tensor_mul(out=st2[:, 1:2], in0=mv[:, 0:1], in1=mv[:, 0:1])
            nc.vector.tensor_add(out=st2[:, 1:2], in0=st2[:, 1:2], in1=mv[:, 1:2])
            nc.vector.tensor_copy(st2[:, 0:1], mv[:, 0:1])
            st_red = small.tile([P, 2], F32)
            nc.gpsimd.partition_all_reduce(
                st_red[:], st2[:], channels=P, reduce_op=bass_isa.ReduceOp.add
            )
            meanv = small.tile([P, 1], F32)
            nc.scalar.mul(out=meanv[:], in_=st_red[:, 0:1], mul=1.0 / P)
            sqm = small.tile([P, 1], F32)
            nc.scalar.mul(out=sqm[:], in_=st_red[:, 1:2], mul=1.0 / P)
            msq = small.tile([P, 1], F32)
            nc.vector.tensor_mul(out=msq[:], in0=meanv[:], in1=meanv[:])
            varv = small.tile([P, 1], F32)
            nc.vector.tensor_sub(out=varv[:], in0=sqm[:], in1=msq[:])
            epsb = small.tile([P, 1], F32)
            nc.vector.memset(epsb[:], EPS)
            stdv = small.tile([P, 1], F32)
            nc.scalar.activation(
                out=stdv[:], in_=varv[:], func=Act.Sqrt, bias=epsb[:], scale=1.0
            )
            rstd = small.tile([P, 1], F32)
            nc.vector.reciprocal(out=rstd[:], in_=stdv[:])
            rs2 = small.tile([P, 1], F32)  # rstd^2 (for the Q fold)
            nc.vector.tensor_mul(out=rs2[:], in0=rstd[:], in1=rstd[:])
            # A = gamma * rstd ; Bterm = beta - mean * A
            A = small.tile([P, CO], F32)
            nc.vector.tensor_scalar_mul(out=A[:], in0=gam[:], scalar1=rstd[:])
            Bt = small.tile([P, CO], F32)
            nc.vector.tensor_scalar_mul(out=Bt[:], in0=A[:], scalar1=meanv[:])
            nc.vector.tensor_sub(out=Bt[:], in0=bet[:], in1=Bt[:])
            Br = small.tile([P, CO], F32R)
            nc.vector.tensor_copy(Br[:], Bt[:])
            # proj weights absorb the V-path rstd factor
            wpjs = wts.tile([P, CO, C], F32R)
            for co in range(CO):
                nc.vector.tensor_scalar_mul(
                    out=wpjs[:, co, :], in0=wpj[:, co, :].bitcast(F32), scalar1=rstd[:]
                )
            ones32 = small.tile([P, P], F32)
            nc.vector.memset(ones32[:], 1.0)
            ones_r = small.tile([P, P], F32R)
            nc.vector.tensor_copy(ones_r[:], ones32[:])

            if _KNOB_STAGE < 2:
                return

            # ---------------- K = gamma-scaled Wk x (raw)  [c, j] ----------------
            # plain rounding copies: k-bias and rstd live on the Q side
            K_sb = big.tile([P, CO, HW], F32R)
            for d2 in range(CO):
                for jt in range(HW // 512):
                    ps = mmp.tile([P, CHW0], F32, tag="qk")
                    for co in range(CO):
                        nc.tensor.matmul(
                            ps[:, 0:512],
                            wqs[:, co, 256 + d2 * P : 256 + (d2 + 1) * P],
                            xps[jt // 2][
                                :, co, (jt % 2) * 512 : (jt % 2 + 1) * 512
                            ],
                            start=(co == 0),
                            stop=(co == CO - 1),
                        )
                    nc.scalar.copy(
                        out=K_sb[:, d2, jt * 512 : (jt + 1) * 512], in_=ps[:, 0:512]
                    )

            if _KNOB_STAGE < 3:
                return

            # ---------------- effective qkv bias (per-partition via bounce) ----
            ps_b1 = mmp.tile([P, CHW0], F32, tag="qk")
            ps_b2 = mmp.tile([P, CHW0], F32, tag="qk")
            for co in range(CO):
                st, sp = (co == 0), (co == CO - 1)
                nc.tensor.matmul(
                    ps_b1[:1, 0:512],
                    Br[:, co : co + 1],
                    wq[:, co, 0:512],
                    start=st,
                    stop=sp,
                )
                nc.tensor.matmul(
                    ps_b2[:1, 0:256],
                    Br[:, co : co + 1],
                    wq[:, co, 512:768],
                    start=st,
                    stop=sp,
                )
            bflat = small.tile([1, 3 * C], F32)
            nc.vector.tensor_add(
                out=bflat[:1, 0:512], in0=ps_b1[:1, 0:512], in1=qb_sb[:1, 0:512]
            )
            nc.vector.tensor_add(
                out=bflat[:1, 512:768], in0=ps_b2[:1, 0:256], in1=qb_sb[:1, 512:768]
            )
            bd = dramp.tile([1, 3 * C], F32)
            nc.scalar.dma_start(bd[:1, :], bflat[:1, :])
            bqk = small.tile([P, 3, CO], F32)
            nc.scalar.dma_start(
                bqk[:], bd[:][0].rearrange("(w co cp) -> cp w co", cp=P, w=3)
            )
            bqs = small.tile([P, CO], F32)  # rstd * bq, the Q-side additive
            nc.vector.tensor_scalar_mul(out=bqs[:], in0=bqk[:, 0, :], scalar1=rstd[:])
            bqs2 = bqs

            if _KNOB_STAGE < 4:
                return

            # ---------------- V^T = x^T (gamma-scaled Wv)  [j, c] ----------------
            NJB = HW // P  # 32 j-blocks
            VT_sb = big.tile([P, NJB, C], F32R)
            for jb in range(NJB):
                ps = mmp.tile([P, CHW0], F32, tag="qk")
                for co in range(CO):
                    nc.tensor.matmul(
                        ps[:, 0:C],
                        xps[jb // 8][:, co, (jb % 8) * P : (jb % 8 + 1) * P],
                        wqs[:, co, 512:768],
                        start=(co == 0),
                        stop=(co == CO - 1),
                    )
                if jb % 2 == 0:
                    nc.scalar.copy(out=VT_sb[:, jb, :], in_=ps[:, 0:C])
                else:
                    nc.vector.tensor_copy(VT_sb[:, jb, :], ps[:, 0:C])

            if _KNOB_STAGE < 5:
                return

            # ------------- Q~ = rstd^2 * (gamma-scaled Wq) xq + rstd*bq  [c, i] -----
            Q_sbs = [
                big.tile([P, CO, CHW0], F32R, tag=f"Qc{k}", name=f"Qc{k}")
                for k in range(NQ // CHW0)
            ]
            for it in range(NQ // 512):
                for d2 in range(CO):
                    ps = mmp.tile([P, CHW0], F32, tag="qk")
                    for co in range(CO):
                        nc.tensor.matmul(
                            ps[:, 0:512],
                            wqs[:, co, d2 * P : (d2 + 1) * P],
                            xqh[it // 2][
                                :, co, (it % 2) * 512 : (it % 2 + 1) * 512
                            ],
                            start=(co == 0),
                            stop=(co == CO - 1),
                        )
                    if d2 == 0:
                        nc.scalar.activation(
                            out=Q_sbs[it // 2][
                                :, d2, (it % 2) * 512 : (it % 2 + 1) * 512
                            ],
                            in_=ps[:, 0:512],
                            func=Act.Identity,
                            bias=bqs2[:, d2 : d2 + 1],
                            scale=rs2[:],
                        )
                    else:
                        nc.vector.tensor_scalar(
                            out=Q_sbs[it // 2][
                                :, d2, (it % 2) * 512 : (it % 2 + 1) * 512
                            ],
                            in0=ps[:, 0:512],
                            scalar1=rs2[:],
                            scalar2=bqs[:, d2 : d2 + 1],
                            op0=Alu.mult,
                            op1=Alu.add,
                        )

            # proj bias absorbs V bias: bpj = projb + Wp @ bv
            bqkv_r = small.tile([P, CO], F32R)
            nc.vector.tensor_copy(bqkv_r[:], bqk[:, 2, :])
            ps_pb = mmp.tile([P, CHW0], F32, tag="qk")
            for co in range(CO):
                nc.tensor.matmul(
                    ps_pb[:1, 0:C],
                    bqkv_r[:, co : co + 1],
                    wpj[:, co, :],
                    start=(co == 0),
                    stop=(co == CO - 1),
                )
            pb_flat = small.tile([1, C], F32)
            nc.vector.tensor_add(
                out=pb_flat[:1, :], in0=ps_pb[:1, 0:C], in1=pb_sb[:1, :]
            )
            bd2 = dramp.tile([1, C], F32)
            nc.scalar.dma_start(bd2[:1, :], pb_flat[:1, :])
            bpj = small.tile([P, CO], F32)
            nc.scalar.dma_start(bpj[:], bd2[:][0].rearrange("(co cp) -> cp co", cp=P))

            if _KNOB_STAGE < 6:
                return

            # ---------------- attention (streamed, transposed) ----------------
            # chunk widths [1024, 512, 512]: wide chunks amortize per-op
            # overheads; the narrow final chunks halve the exposed tail chain
            CHUNKS = [(0, 1024), (1024, 512), (1536, 512)]
            ao_sbs = [
                big.tile([P, CO, cw], F32R, tag=f"aoc{k}", name=f"aoc{k}")
                for k, (_, cw) in enumerate(CHUNKS)
            ]

            def _emit_proj(pci):
                coff, cw = CHUNKS[pci]
                for i2 in range(cw // 512):
                    for d2 in range(CO):
                        s = slice(coff + i2 * 512, coff + (i2 + 1) * 512)
                        pps = mmp.tile([P, CHW0], F32, tag="qk", name="pps")
                        for co in range(CO):
                            nc.tensor.matmul(
                                pps[:, 0:512],
                                wpjs[:, co, d2 * P : (d2 + 1) * P],
                                ao_sbs[pci][:, co, i2 * 512 : (i2 + 1) * 512],
                                start=(co == 0),
                                stop=(co == CO - 1),
                            )
                        ot = outp.tile([P, 512], F32, tag="ot", name="ot")
                        nc.scalar.activation(
                            out=ot[:],
                            in_=pps[:, 0:512],
                            func=Act.Identity,
                            bias=bpj[:, d2 : d2 + 1],
                            scale=1.0,
                        )
                        g0 = coff + i2 * 512
                        yt = outp.tile([P, 512], F32, tag="yt", name="yt")
                        nc.vector.tensor_add(
                            out=yt[:],
                            in0=ot[:],
                            in1=xqh[g0 // 1024][
                                :, d2, g0 % 1024 : g0 % 1024 + 512
                            ].bitcast(F32),
                        )
                        nc.sync.dma_start(_co_view(y[:])[:, d2, s], yt[:])

            for ci, (coff, cw) in enumerate(CHUNKS):
                ni2 = cw // 512
                qi, qoff = coff // CHW0, coff % CHW0
                # denominator partials: even j-blocks on DVE, odd on GpSimd
                dacc_v = dap.tile([P, CHW0], F32R, tag="dacc_v")
                nc.vector.memset(dacc_v[:, 0:cw].bitcast(F32), 0.0)
                dacc_g = dap.tile([P, CHW0], F32, tag="dacc_g")
                nc.gpsimd.memset(dacc_g[:, 0:cw], 0.0)
                pvs = [
                    pvp.tile([P, 512], F32, tag=f"pv{k}", name=f"pv{k}")
                    for k in range(CO * ni2)
                ]
                exs = [None] * NJB
                # software-pipelined 2-deep: QK/exp for jb before PV for jb-2
                for jb in range(NJB + 2):
                    if jb < NJB:
                        qk = mmp.tile([P, CHW0], F32, tag="qk")
                        for i2 in range(ni2):
                            for co in range(CO):
                                nc.tensor.matmul(
                                    qk[:, i2 * 512 : (i2 + 1) * 512],
                                    K_sb[:, co, jb * P : (jb + 1) * P],
                                    Q_sbs[qi][
                                        :, co, qoff + i2 * 512 : qoff + (i2 + 1) * 512
                                    ],
                                    start=(co == 0),
                                    stop=(co == CO - 1),
                                )
                        ex = expp.tile([P, CHW0], F32R, tag="ex")
                        nc.scalar.activation(
                            out=ex[:, 0:cw], in_=qk[:, 0:cw], func=Act.Exp, scale=SCALE
                        )
                        if jb % 2 == 0:
                            nc.vector.tensor_add(
                                out=dacc_v[:, 0:cw],
                                in0=dacc_v[:, 0:cw].bitcast(F32),
                                in1=ex[:, 0:cw].bitcast(F32),
                            )
                        else:
                            nc.gpsimd.tensor_add(
                                out=dacc_g[:, 0:cw],
                                in0=dacc_g[:, 0:cw],
                                in1=ex[:, 0:cw].bitcast(F32),
                            )
                        exs[jb] = ex
                    if jb >= 2:
                        pj = jb - 2
                        for c2 in range(CO):
                            for i2 in range(ni2):
                                nc.tensor.matmul(
                                    pvs[c2 * ni2 + i2][:],
                                    VT_sb[:, pj, c2 * P : (c2 + 1) * P],
                                    exs[pj][:, i2 * 512 : (i2 + 1) * 512],
                                    start=(pj == 0),
                                    stop=(pj == NJB - 1),
                                )
                    if jb == 6 and ci > 0:
                        _emit_proj(ci - 1)

                dgr = dap.tile([P, CHW0], F32R, tag="dgr")
                nc.vector.tensor_copy(dgr[:, 0:cw], dacc_g[:, 0:cw])
                dps = mmp.tile([P, CHW0], F32, tag="qk")
                for i2 in range(ni2):
                    s = slice(i2 * 512, (i2 + 1) * 512)
                    nc.tensor.matmul(
                        dps[:, s], ones_r[:], dacc_v[:, s], start=True, stop=False
                    )
                    nc.tensor.matmul(
                        dps[:, s], ones_r[:], dgr[:, s], start=False, stop=True
                    )
                rec = dap.tile([P, CHW0], F32, tag="rec")
                for i2 in range(ni2):
                    si = slice(i2 * 512, (i2 + 1) * 512)
                    nc.vector.reciprocal(out=rec[:, si], in_=dps[:, si])
                    for c2 in range(CO):
                        nc.vector.tensor_mul(
                            out=ao_sbs[ci][:, c2, si],
                            in0=pvs[c2 * ni2 + i2][:],
                            in1=rec[:, si],
                        )
                if ci == len(CHUNKS) - 1:
                    _emit_proj(ci)



def _get_prog():
    global _prog
    if _prog is None:
        _prog = _build()
    return _prog


def kernel(x, norm_gamma, norm_beta, qkv_w, qkv_b, proj_w, proj_b, **_ignored):
    x = np.ascontiguousarray(np.asarray(x, dtype=np.float32))
    gamma = np.ascontiguousarray(np.asarray(norm_gamma, dtype=np.float32))
    beta = np.ascontiguousarray(np.asarray(norm_beta, dtype=np.float32))
    qkv_b = np.ascontiguousarray(np.asarray(qkv_b, dtype=np.float32))
    proj_b = np.ascontiguousarray(np.asarray(proj_b, dtype=np.float32))
    wqkvT = np.ascontiguousarray(np.asarray(qkv_w, dtype=np.float32).T)
    wprojT = np.ascontiguousarray(np.asarray(proj_w, dtype=np.float32).T)

    nc = _get_prog()
    in_maps = []
    for c in range(NCORES):
        b, h = divmod(c, 2)
        xb = np.ascontiguousarray(x[b].reshape(C, HW))
        xqh = np.ascontiguousarray(xb[:, h * NQ : (h + 1) * NQ])
        in_maps.append(
            dict(
                xr=xb,
                xq=xqh,
                wqkvT=wqkvT,
                wprojT=wprojT,
                qkvb=qkv_b.reshape(1, 3 * C),
                projb=proj_b.reshape(1, C),
                gamma=gamma.reshape(1, C),
                beta=beta.reshape(1, C),
            )
        )
    res = run_bass_kernel_spmd(nc, in_maps, core_ids=list(range(NCORES)))
    out = np.empty((B, C, HW), np.float32)
    for c in range(NCORES):
        b, h = divmod(c, 2)
        out[b, :, h * NQ : (h + 1) * NQ] = res.results[c]["y"]
    return out.reshape(B, C, 64, 64)


# revision 50
# speedup vs baseline: 1.0355x; 1.0355x over previous
"""AttentionBlock (GroupNorm + single-head 1x1-conv attention + residual) on 8 trn2 cores.

Sharding: 8 cores = (batch b in 0..3) x (query-half h in 0..1). Each core computes
the full attention rows for its 2048 query positions of its sample.

Per-core pipeline (all matmuls in float32r: full PE rate for moving-dim >= 256,
near-fp32 precision in practice):
  1. x loads in pieces; GroupNorm(1 group) stats via bn_stats overlapped with the
     DMA; the cross-partition combine runs on GpSimd (partition_all_reduce) so
     the PE instruction stream is never blocked by statistics.
  2. qkv weights are scaled by gamma only (known immediately), so all QKV
     matmuls start as soon as x pieces land. The scalar rstd and the effective
     q-bias are folded into Q alone: Q~ = rstd^2*q_raw + rstd*bq. Then
     logits = Q~^T K_raw reproduces rstd^2*q.k + u[j]; the k-bias and the
     constant terms cancel inside softmax. K and V therefore get plain
     (stats-free) PSUM->SBUF rounding copies. The V-path rstd is folded into
     the proj weights, and the V bias exactly into the proj bias (softmax rows
     sum to 1): out = (rstd*Wp) @ ao_raw + (Wp @ bv + pb).
  3. Q[c, i] (own half), K[c, j] (full), V^T[j, c] (full, produced transposed
     directly by swapping matmul operands).
  4. Streamed attention over 32 j-blocks x 1024-wide i-chunks: logits^T[jb, i]
     = K_blk^T Q~ into a 2-bank PSUM tile, one fused scale+exp on ScalarE per
     block, denominator partials alternating VectorE/GpSimd (no port contention
     at 1x), PV accumulated over j-blocks in PSUM. Softmax max-subtraction is
     skipped: logits are bounded (|l| <= 16) so exp is safe in fp32 and the
     result is mathematically identical.
  5. Denominator replicated across partitions with a ones-matmul, reciprocal,
     applied to PV. proj + bias + residual per chunk, overlapping the next
     chunk's attention.
"""

import os

import numpy as np

import concourse.bass as bass
import concourse.bass_isa as bass_isa
import concourse.mybir as mybir
import concourse.tile as tile
from concourse import bacc
from concourse.bass_utils import run_bass_kernel_spmd

# perf-experiment knobs (only read at build time; defaults = production)
_KNOB_NCH = int(os.environ.get("K_NCH", "2"))  # attention i-chunks emitted
_KNOB_MMBUFS = int(os.environ.get("K_MMBUFS", "2"))
_KNOB_EXBUFS = int(os.environ.get("K_EXBUFS", "3"))
_KNOB_STAGE = int(os.environ.get("K_STAGE", "9"))

B, C, HW = 4, 256, 4096
CHW0 = 1024  # attention i-chunk width == qk psum tile width (2 banks)
P, CO = 128, 2
NQ = HW // 2  # queries per core
NCORES = 8
SCALE = float(C) ** -0.5
EPS = 1e-5
F32, F32R = mybir.dt.float32, mybir.dt.float32r
Act = mybir.ActivationFunctionType
Alu = mybir.AluOpType

_prog = None


def _co_view(ap):
    """[C, N] dram AP -> [128, 2, N] (partition cp, free (co, n)); c = co*128+cp."""
    return ap.rearrange("(co cp) n -> cp co n", cp=P)


def _build():
    nc = bacc.Bacc(None, target_bir_lowering=False)
    _build_body(nc)
    nc.compile()
    return nc


def _build_body(nc):
    xr = nc.dram_tensor("xr", [C, HW], F32, kind="ExternalInput")
    xq = nc.dram_tensor("xq", [C, NQ], F32, kind="ExternalInput")
    wqkvT = nc.dram_tensor("wqkvT", [C, 3 * C], F32, kind="ExternalInput")
    wprojT = nc.dram_tensor("wprojT", [C, C], F32, kind="ExternalInput")
    qkvb = nc.dram_tensor("qkvb", [1, 3 * C], F32, kind="ExternalInput")
    projb = nc.dram_tensor("projb", [1, C], F32, kind="ExternalInput")
    gamma = nc.dram_tensor("gamma", [1, C], F32, kind="ExternalInput")
    beta = nc.dram_tensor("beta", [1, C], F32, kind="ExternalInput")
    y = nc.dram_tensor("y", [C, NQ], F32, kind="ExternalOutput")

    with tile.TileContext(nc) as tc:
        with (
            tc.tile_pool(name="big", bufs=1) as big,
            tc.tile_pool(name="wts", bufs=1) as wts,
            tc.tile_pool(name="small", bufs=1) as small,
            tc.tile_pool(name="expp", bufs=_KNOB_EXBUFS) as expp,
            tc.tile_pool(name="dap", bufs=1) as dap,
            tc.tile_pool(name="outp", bufs=2) as outp,
            tc.tile_pool(name="mm", bufs=_KNOB_MMBUFS, space="PSUM") as mmp,
            tc.tile_pool(name="pv", bufs=1, space="PSUM") as pvp,
            tc.tile_pool(name="dram", bufs=1, space="DRAM") as dramp,
        ):
            # ---- x load as 4 separate piece tiles (whole-tile dep tracking:
            # separate tiles let bn/K/VT start as each piece lands) ----
            NXP = 4
            XPW = HW // NXP  # 1024
            xps = []
            stt = small.tile([P, 16, 6], F32)
            for p_ in range(NXP):
                nsl = slice(p_ * XPW, (p_ + 1) * XPW)
                xp = big.tile([P, CO, XPW], F32R, tag=f"xp{p_}", name=f"xp{p_}")
                xps.append(xp)
                nc.sync.dma_start(xp[:], _co_view(xr[:])[:, :, nsl].bitcast(F32R))
                for co in range(CO):
                    for s2 in range(XPW // 512):
                        nc.vector.bn_stats(
                            out=stt[:, p_ * 4 + co * 2 + s2, :],
                            in_=xp[:, co, s2 * 512 : (s2 + 1) * 512].bitcast(F32),
                        )
            # xq rides the same (sync) HWDGE ring so it starts after the
            # stats-critical xr pieces have drained; split in half so chunk 0's
            # Q matmuls start a transfer earlier
            xqh = []
            for hh in range(2):
                t = big.tile([P, CO, NQ // 2], F32R, tag=f"xqh{hh}", name=f"xqh{hh}")
                xqh.append(t)
                nc.sync.dma_start(
                    t[:],
                    _co_view(xq[:])[:, :, hh * (NQ // 2) : (hh + 1) * (NQ // 2)]
                    .bitcast(F32R),
                )

            # ---- weights (gamma-only scale: no stats dependency) ----
            # wq first on the scalar HWDGE ring: it gates the first K matmuls
            wq = wts.tile([P, CO, 3 * C], F32R)
            nc.scalar.dma_start(wq[:], _co_view(wqkvT[:]).bitcast(F32R))
            gam = small.tile([P, CO], F32)
            nc.scalar.dma_start(gam[:], gamma[:][0].rearrange("(co cp) -> cp co", cp=P))
            bet = small.tile([P, CO], F32)
            nc.scalar.dma_start(bet[:], beta[:][0].rearrange("(co cp) -> cp co", cp=P))
            wqs = wts.tile([P, CO, 3 * C], F32R)
            for co in range(CO):
                nc.vector.tensor_scalar_mul(
                    out=wqs[:, co, :],
                    in0=wq[:, co, :].bitcast(F32),
                    scalar1=gam[:, co : co + 1],
                )
            wpj = wts.tile([P, CO, C], F32R)
            nc.scalar.dma_start(wpj[:], _co_view(wprojT[:]).bitcast(F32R))
            qb_sb = small.tile([1, 3 * C], F32)
            nc.scalar.dma_start(qb_sb[:1, :], qkvb[:])
            pb_sb = small.tile([1, C], F32)
            nc.scalar.dma_start(pb_sb[:1, :], projb[:])

            # ---- groupnorm stats (DVE + GpSimd only; PE never stalls) ----
            mv = small.tile([P, 2], F32)
            nc.vector.bn_aggr(out=mv[:], in_=stt[:])
            st2 = small.tile([P, 2], F32)
            nc.vector.tensor_mul(out=st2[:, 1:2], in0=mv[:, 0:1], in1=mv[:, 0:1])
            nc.vector.tensor_add(out=st2[:, 1:2], in0=st2[:, 1:2], in1=mv[:, 1:2])
            nc.vector.tensor_copy(st2[:, 0:1], mv[:, 0:1])
            st_red = small.tile([P, 2], F32)
            nc.gpsimd.partition_all_reduce(
                st_red[:], st2[:], channels=P, reduce_op=bass_isa.ReduceOp.add
            )
            meanv = small.tile([P, 1], F32)
            nc.scalar.mul(out=meanv[:], in_=st_red[:, 0:1], mul=1.0 / P)
            sqm = small.tile([P, 1], F32)
            nc.scalar.mul(out=sqm[:], in_=st_red[:, 1:2], mul=1.0 / P)
            msq = small.tile([P, 1], F32)
            nc.vector.tensor_mul(out=msq[:], in0=meanv[:], in1=meanv[:])
            varv = small.tile([P, 1], F32)
            nc.vector.tensor_sub(out=varv[:], in0=sqm[:], in1=msq[:])
            epsb = small.tile([P, 1], F32)
            nc.vector.memset(epsb[:], EPS)
            stdv = small.tile([P, 1], F32)
            nc.scalar.activation(
                out=stdv[:], in_=varv[:], func=Act.Sqrt, bias=epsb[:], scale=1.0
            )
            rstd = small.tile([P, 1], F32)
            nc.vector.reciprocal(out=rstd[:], in_=stdv[:])
            rs2 = small.tile([P, 1], F32)  # rstd^2 (for the Q fold)
            nc.vector.tensor_mul(out=rs2[:], in0=rstd[:], in1=rstd[:])
            # A = gamma * rstd ; Bterm = beta - mean * A
            A = small.tile([P, CO], F32)
            nc.vector.tensor_scalar_mul(out=A[:], in0=gam[:], scalar1=rstd[:])
            Bt = small.tile([P, CO], F32)
            nc.vector.tensor_scalar_mul(out=Bt[:], in0=A[:], scalar1=meanv[:])
            nc.vector.tensor_sub(out=Bt[:], in0=bet[:], in1=Bt[:])
            Br = small.tile([P, CO], F32R)
            nc.vector.tensor_copy(Br[:], Bt[:])
            # proj weights absorb the V-path rstd factor
            wpjs = wts.tile([P, CO, C], F32R)
            for co in range(CO):
                nc.vector.tensor_scalar_mul(
                    out=wpjs[:, co, :], in0=wpj[:, co, :].bitcast(F32), scalar1=rstd[:]
                )
            ones32 = small.tile([P, P], F32)
            nc.vector.memset(ones32[:], 1.0)
            ones_r = small.tile([P, P], F32R)
            nc.vector.tensor_copy(ones_r[:], ones32[:])

            if _KNOB_STAGE < 2:
                return

            # ---------------- K = gamma-scaled Wk x (raw)  [c, j] ----------------
            # plain rounding copies: k-bias and rstd live on the Q side
            K_sb = big.tile([P, CO, HW], F32R)
            for d2 in range(CO):
                for jp in range(HW // 1024):
                    ps = mmp.tile([P, CHW0], F32, tag="qk")
                    for half in range(2):
                        jt = jp * 2 + half
                        for co in range(CO):
                            nc.tensor.matmul(
                                ps[:, half * 512 : (half + 1) * 512],
                                wqs[:, co, 256 + d2 * P : 256 + (d2 + 1) * P],
                                xps[jt // 2][
                                    :, co, (jt % 2) * 512 : (jt % 2 + 1) * 512
                                ],
                                start=(co == 0),
                                stop=(co == CO - 1),
                            )
                    nc.scalar.copy(
                        out=K_sb[:, d2, jp * 1024 : (jp + 1) * 1024], in_=ps[:]
                    )

            if _KNOB_STAGE < 3:
                return

            # ---------------- effective qkv bias (per-partition via bounce) ----
            ps_b1 = mmp.tile([P, CHW0], F32, tag="qk")
            ps_b2 = mmp.tile([P, CHW0], F32, tag="qk")
            for co in range(CO):
                st, sp = (co == 0), (co == CO - 1)
                nc.tensor.matmul(
                    ps_b1[:1, 0:512],
                    Br[:, co : co + 1],
                    wq[:, co, 0:512],
                    start=st,
                    stop=sp,
                )
                nc.tensor.matmul(
                    ps_b2[:1, 0:256],
                    Br[:, co : co + 1],
                    wq[:, co, 512:768],
                    start=st,
                    stop=sp,
                )
            bflat = small.tile([1, 3 * C], F32)
            nc.vector.tensor_add(
                out=bflat[:1, 0:512], in0=ps_b1[:1, 0:512], in1=qb_sb[:1, 0:512]
            )
            nc.vector.tensor_add(
                out=bflat[:1, 512:768], in0=ps_b2[:1, 0:256], in1=qb_sb[:1, 512:768]
            )
            bd = dramp.tile([1, 3 * C], F32)
            nc.scalar.dma_start(bd[:1, :], bflat[:1, :])
            bqk = small.tile([P, 3, CO], F32)
            nc.scalar.dma_start(
                bqk[:], bd[:][0].rearrange("(w co cp) -> cp w co", cp=P, w=3)
            )
            bqs = small.tile([P, CO], F32)  # rstd * bq, the Q-side additive
            nc.vector.tensor_scalar_mul(out=bqs[:], in0=bqk[:, 0, :], scalar1=rstd[:])
            bqs2 = bqs

            if _KNOB_STAGE < 4:
                return

            # ---------------- V^T = x^T (gamma-scaled Wv)  [j, c] ----------------
            NJB = HW // P  # 32 j-blocks
            VT_sb = big.tile([P, NJB, C], F32R)
            for jg in range(NJB // 4):
                ps = mmp.tile([P, CHW0], F32, tag="qk")
                for k4 in range(4):
                    jb = jg * 4 + k4
                    for co in range(CO):
                        nc.tensor.matmul(
                            ps[:, k4 * C : (k4 + 1) * C],
                            xps[jb // 8][:, co, (jb % 8) * P : (jb % 8 + 1) * P],
                            wqs[:, co, 512:768],
                            start=(co == 0),
                            stop=(co == CO - 1),
                        )
                if jg % 2 == 0:
                    nc.scalar.copy(
                        out=VT_sb[:, jg * 4 : (jg + 1) * 4, :], in_=ps[:]
                    )
                else:
                    nc.vector.tensor_copy(
                        VT_sb[:, jg * 4 : (jg + 1) * 4, :], ps[:]
                    )

            if _KNOB_STAGE < 5:
                return

            # ------------- Q~ = rstd^2 * (gamma-scaled Wq) xq + rstd*bq  [c, i] -----
            Q_sbs = [
                big.tile([P, CO, CHW0], F32R, tag=f"Qc{k}", name=f"Qc{k}")
                for k in range(NQ // CHW0)
            ]
            for it in range(NQ // 512):
                for d2 in range(CO):
                    ps = mmp.tile([P, CHW0], F32, tag="qk")
                    for co in range(CO):
                        nc.tensor.matmul(
                            ps[:, 0:512],
                            wqs[:, co, d2 * P : (d2 + 1) * P],
                            xqh[it // 2][
                                :, co, (it % 2) * 512 : (it % 2 + 1) * 512
                            ],
                            start=(co == 0),
                            stop=(co == CO - 1),
                        )
                    if d2 == 0:
                        nc.scalar.activation(
                            out=Q_sbs[it // 2][
                                :, d2, (it % 2) * 512 : (it % 2 + 1) * 512
                            ],
                            in_=ps[:, 0:512],
                            func=Act.Identity,
                            bias=bqs2[:, d2 : d2 + 1],
                            scale=rs2[:],
                        )
                    else:
                        nc.vector.tensor_scalar(
                            out=Q_sbs[it // 2][
                                :, d2, (it % 2) * 512 : (it % 2 + 1) * 512
                            ],
                            in0=ps[:, 0:512],
                            scalar1=rs2[:],
                            scalar2=bqs[:, d2 : d2 + 1],
                            op0=Alu.mult,
                            op1=Alu.add,
                        )

            # proj bias absorbs V bias: bpj = projb + Wp @ bv
            bqkv_r = small.tile([P, CO], F32R)
            nc.vector.tensor_copy(bqkv_r[:], bqk[:, 2, :])
            ps_pb = mmp.tile([P, CHW0], F32, tag="qk")
            for co in range(CO):
                nc.tensor.matmul(
                    ps_pb[:1, 0:C],
                    bqkv_r[:, co : co + 1],
                    wpj[:, co, :],
                    start=(co == 0),
                    stop=(co == CO - 1),
                )
            pb_flat = small.tile([1, C], F32)
            nc.vector.tensor_add(
                out=pb_flat[:1, :], in0=ps_pb[:1, 0:C], in1=pb_sb[:1, :]
            )
            bd2 = dramp.tile([1, C], F32)
            nc.scalar.dma_start(bd2[:1, :], pb_flat[:1, :])
            bpj = small.tile([P, CO], F32)
            nc.scalar.dma_start(bpj[:], bd2[:][0].rearrange("(co cp) -> cp co", cp=P))

            if _KNOB_STAGE < 6:
                return

            # ---------------- attention (streamed, transposed) ----------------
            # chunk widths [1024, 512, 512]: wide chunks amortize per-op
            # overheads; the narrow final chunks halve the exposed tail chain
            CHUNKS = [(0, 1024), (1024, 512), (1536, 512)]
            ao_sbs = [
                big.tile([P, CO, cw], F32R, tag=f"aoc{k}", name=f"aoc{k}")
                for k, (_, cw) in enumerate(CHUNKS)
            ]

            def _emit_proj(pci):
                coff, cw = CHUNKS[pci]
                for i2 in range(cw // 512):
                    for d2 in range(CO):
                        s = slice(coff + i2 * 512, coff + (i2 + 1) * 512)
                        pps = mmp.tile([P, CHW0], F32, tag="qk", name="pps")
                        for co in range(CO):
                            nc.tensor.matmul(
                                pps[:, 0:512],
                                wpjs[:, co, d2 * P : (d2 + 1) * P],
                                ao_sbs[pci][:, co, i2 * 512 : (i2 + 1) * 512],
                                start=(co == 0),
                                stop=(co == CO - 1),
                            )
                        ot = outp.tile([P, 512], F32, tag="ot", name="ot")
                        nc.scalar.activation(
                            out=ot[:],
                            in_=pps[:, 0:512],
                            func=Act.Identity,
                            bias=bpj[:, d2 : d2 + 1],
                            scale=1.0,
                        )
                        g0 = coff + i2 * 512
                        yt = outp.tile([P, 512], F32, tag="yt", name="yt")
                        nc.vector.tensor_add(
                            out=yt[:],
                            in0=ot[:],
                            in1=xqh[g0 // 1024][
                                :, d2, g0 % 1024 : g0 % 1024 + 512
                            ].bitcast(F32),
                        )
                        nc.sync.dma_start(_co_view(y[:])[:, d2, s], yt[:])

            for ci, (coff, cw) in enumerate(CHUNKS):
                ni2 = cw // 512
                qi, qoff = coff // CHW0, coff % CHW0
                # denominator partials: even j-blocks on DVE, odd on GpSimd
                dacc_v = dap.tile([P, CHW0], F32R, tag="dacc_v")
                nc.vector.memset(dacc_v[:, 0:cw].bitcast(F32), 0.0)
                dacc_g = dap.tile([P, CHW0], F32, tag="dacc_g")
                nc.gpsimd.memset(dacc_g[:, 0:cw], 0.0)
                pvs = [
                    pvp.tile([P, 512], F32, tag=f"pv{k}", name=f"pv{k}")
                    for k in range(CO * ni2)
                ]
                exs = [None] * NJB
                # software-pipelined 2-deep: QK/exp for jb before PV for jb-2
                for jb in range(NJB + 2):
                    if jb < NJB:
                        qk = mmp.tile([P, CHW0], F32, tag="qk")
                        for i2 in range(ni2):
                            for co in range(CO):
                                nc.tensor.matmul(
                                    qk[:, i2 * 512 : (i2 + 1) * 512],
                                    K_sb[:, co, jb * P : (jb + 1) * P],
                                    Q_sbs[qi][
                                        :, co, qoff + i2 * 512 : qoff + (i2 + 1) * 512
                                    ],
                                    start=(co == 0),
                                    stop=(co == CO - 1),
                                )
                        ex = expp.tile([P, CHW0], F32R, tag="ex")
                        nc.scalar.activation(
                            out=ex[:, 0:cw], in_=qk[:, 0:cw], func=Act.Exp, scale=SCALE
                        )
                        if jb % 2 == 0:
                            nc.vector.tensor_add(
                                out=dacc_v[:, 0:cw],
                                in0=dacc_v[:, 0:cw].bitcast(F32),
                                in1=ex[:, 0:cw].bitcast(F32),
                            )
                        else:
                            nc.gpsimd.tensor_add(
                                out=dacc_g[:, 0:cw],
                                in0=dacc_g[:, 0:cw],
                                in1=ex[:, 0:cw].bitcast(F32),
                            )
                        exs[jb] = ex
                    if jb >= 2:
                        pj = jb - 2
                        for c2 in range(CO):
                            for i2 in range(ni2):
                                nc.tensor.matmul(
                                    pvs[c2 * ni2 + i2][:],
                                    VT_sb[:, pj, c2 * P : (c2 + 1) * P],
                                    exs[pj][:, i2 * 512 : (i2 + 1) * 512],
                                    start=(pj == 0),
                                    stop=(pj == NJB - 1),
                                )
                    if jb == 6 and ci > 0:
                        _emit_proj(ci - 1)

                dgr = dap.tile([P, CHW0], F32R, tag="dgr")
                nc.vector.tensor_copy(dgr[:, 0:cw], dacc_g[:, 0:cw])
                dps = mmp.tile([P, CHW0], F32, tag="qk")
                for i2 in range(ni2):
                    s = slice(i2 * 512, (i2 + 1) * 512)
                    nc.tensor.matmul(
                        dps[:, s], ones_r[:], dacc_v[:, s], start=True, stop=False
                    )
                    nc.tensor.matmul(
                        dps[:, s], ones_r[:], dgr[:, s], start=False, stop=True
                    )
                rec = dap.tile([P, CHW0], F32, tag="rec")
                for i2 in range(ni2):
                    si = slice(i2 * 512, (i2 + 1) * 512)
                    nc.vector.reciprocal(out=rec[:, si], in_=dps[:, si])
                    for c2 in range(CO):
                        nc.vector.tensor_mul(
                            out=ao_sbs[ci][:, c2, si],
                            in0=pvs[c2 * ni2 + i2][:],
                            in1=rec[:, si],
                        )
                if ci == len(CHUNKS) - 1:
                    _emit_proj(ci)



def _get_prog():
    global _prog
    if _prog is None:
        _prog = _build()
    return _prog


def kernel(x, norm_gamma, norm_beta, qkv_w, qkv_b, proj_w, proj_b, **_ignored):
    x = np.ascontiguousarray(np.asarray(x, dtype=np.float32))
    gamma = np.ascontiguousarray(np.asarray(norm_gamma, dtype=np.float32))
    beta = np.ascontiguousarray(np.asarray(norm_beta, dtype=np.float32))
    qkv_b = np.ascontiguousarray(np.asarray(qkv_b, dtype=np.float32))
    proj_b = np.ascontiguousarray(np.asarray(proj_b, dtype=np.float32))
    wqkvT = np.ascontiguousarray(np.asarray(qkv_w, dtype=np.float32).T)
    wprojT = np.ascontiguousarray(np.asarray(proj_w, dtype=np.float32).T)

    nc = _get_prog()
    in_maps = []
    for c in range(NCORES):
        b, h = divmod(c, 2)
        xb = np.ascontiguousarray(x[b].reshape(C, HW))
        xqh = np.ascontiguousarray(xb[:, h * NQ : (h + 1) * NQ])
        in_maps.append(
            dict(
                xr=xb,
                xq=xqh,
                wqkvT=wqkvT,
                wprojT=wprojT,
                qkvb=qkv_b.reshape(1, 3 * C),
                projb=proj_b.reshape(1, C),
                gamma=gamma.reshape(1, C),
                beta=beta.reshape(1, C),
            )
        )
    res = run_bass_kernel_spmd(nc, in_maps, core_ids=list(range(NCORES)))
    out = np.empty((B, C, HW), np.float32)
    for c in range(NCORES):
        b, h = divmod(c, 2)
        out[b, :, h * NQ : (h + 1) * NQ] = res.results[c]["y"]
    return out.reshape(B, C, 64, 64)


# revision 52
# speedup vs baseline: 1.0484x; 1.0124x over previous
"""AttentionBlock (GroupNorm + single-head 1x1-conv attention + residual) on 8 trn2 cores.

Sharding: 8 cores = (batch b in 0..3) x (query-half h in 0..1). Each core computes
the full attention rows for its 2048 query positions of its sample.

Per-core pipeline (all matmuls in float32r: full PE rate for moving-dim >= 256,
near-fp32 precision in practice):
  1. x loads in pieces; GroupNorm(1 group) stats via bn_stats overlapped with the
     DMA; the cross-partition combine runs on GpSimd (partition_all_reduce) so
     the PE instruction stream is never blocked by statistics.
  2. qkv weights are scaled by gamma only (known immediately), so all QKV
     matmuls start as soon as x pieces land. The scalar rstd and the effective
     q-bias are folded into Q alone: Q~ = rstd^2*q_raw + rstd*bq. Then
     logits = Q~^T K_raw reproduces rstd^2*q.k + u[j]; the k-bias and the
     constant terms cancel inside softmax. K and V therefore get plain
     (stats-free) PSUM->SBUF rounding copies. The V-path rstd is folded into
     the proj weights, and the V bias exactly into the proj bias (softmax rows
     sum to 1): out = (rstd*Wp) @ ao_raw + (Wp @ bv + pb).
  3. Q[c, i] (own half), K[c, j] (full), V^T[j, c] (full, produced transposed
     directly by swapping matmul operands).
  4. Streamed attention over 32 j-blocks x 1024-wide i-chunks: logits^T[jb, i]
     = K_blk^T Q~ into a 2-bank PSUM tile, one fused scale+exp on ScalarE per
     block, denominator partials alternating VectorE/GpSimd (no port contention
     at 1x), PV accumulated over j-blocks in PSUM. Softmax max-subtraction is
     skipped: logits are bounded (|l| <= 16) so exp is safe in fp32 and the
     result is mathematically identical.
  5. Denominator replicated across partitions with a ones-matmul, reciprocal,
     applied to PV. proj + bias + residual per chunk, overlapping the next
     chunk's attention.
"""

import os

import numpy as np

import concourse.bass as bass
import concourse.bass_isa as bass_isa
import concourse.mybir as mybir
import concourse.tile as tile
from concourse import bacc
from concourse.bass_utils import run_bass_kernel_spmd

# perf-experiment knobs (only read at build time; defaults = production)
_KNOB_NCH = int(os.environ.get("K_NCH", "2"))  # attention i-chunks emitted
_KNOB_MMBUFS = int(os.environ.get("K_MMBUFS", "2"))
_KNOB_EXBUFS = int(os.environ.get("K_EXBUFS", "4"))
_KNOB_STAGE = int(os.environ.get("K_STAGE", "9"))

B, C, HW = 4, 256, 4096
CHW0 = 1024  # attention i-chunk width == qk psum tile width (2 banks)
P, CO = 128, 2
NQ = HW // 2  # queries per core
NCORES = 8
SCALE = float(C) ** -0.5
EPS = 1e-5
F32, F32R = mybir.dt.float32, mybir.dt.float32r
Act = mybir.ActivationFunctionType
Alu = mybir.AluOpType

_prog = None


def _co_view(ap):
    """[C, N] dram AP -> [128, 2, N] (partition cp, free (co, n)); c = co*128+cp."""
    return ap.rearrange("(co cp) n -> cp co n", cp=P)


def _build():
    nc = bacc.Bacc(None, target_bir_lowering=False)
    _build_body(nc)
    nc.compile()
    return nc


def _build_body(nc):
    xr = nc.dram_tensor("xr", [C, HW], F32, kind="ExternalInput")
    xq = nc.dram_tensor("xq", [C, NQ], F32, kind="ExternalInput")
    wqkvT = nc.dram_tensor("wqkvT", [C, 3 * C], F32, kind="ExternalInput")
    wprojT = nc.dram_tensor("wprojT", [C, C], F32, kind="ExternalInput")
    qkvb = nc.dram_tensor("qkvb", [1, 3 * C], F32, kind="ExternalInput")
    projb = nc.dram_tensor("projb", [1, C], F32, kind="ExternalInput")
    gamma = nc.dram_tensor("gamma", [1, C], F32, kind="ExternalInput")
    beta = nc.dram_tensor("beta", [1, C], F32, kind="ExternalInput")
    y = nc.dram_tensor("y", [C, NQ], F32, kind="ExternalOutput")

    with tile.TileContext(nc) as tc:
        with (
            tc.tile_pool(name="big", bufs=1) as big,
            tc.tile_pool(name="wts", bufs=1) as wts,
            tc.tile_pool(name="small", bufs=1) as small,
            tc.tile_pool(name="expp", bufs=_KNOB_EXBUFS) as expp,
            tc.tile_pool(name="dap", bufs=1) as dap,
            tc.tile_pool(name="outp", bufs=2) as outp,
            tc.tile_pool(name="mm", bufs=_KNOB_MMBUFS, space="PSUM") as mmp,
            tc.tile_pool(name="pv", bufs=1, space="PSUM") as pvp,
            tc.tile_pool(name="dram", bufs=1, space="DRAM") as dramp,
        ):
            # ---- x load as 4 separate piece tiles (whole-tile dep tracking:
            # separate tiles let bn/K/VT start as each piece lands) ----
            NXP = 4
            XPW = HW // NXP  # 1024
            xps = []
            stt = small.tile([P, 16, 6], F32)
            for p_ in range(NXP):
                nsl = slice(p_ * XPW, (p_ + 1) * XPW)
                xp = big.tile([P, CO, XPW], F32R, tag=f"xp{p_}", name=f"xp{p_}")
                xps.append(xp)
                nc.sync.dma_start(xp[:], _co_view(xr[:])[:, :, nsl].bitcast(F32R))
                for co in range(CO):
                    for s2 in range(XPW // 512):
                        nc.vector.bn_stats(
                            out=stt[:, p_ * 4 + co * 2 + s2, :],
                            in_=xp[:, co, s2 * 512 : (s2 + 1) * 512].bitcast(F32),
                        )
            # xq rides the same (sync) HWDGE ring so it starts after the
            # stats-critical xr pieces have drained; split in half so chunk 0's
            # Q matmuls start a transfer earlier
            xqh = []
            for hh in range(2):
                t = big.tile([P, CO, NQ // 2], F32R, tag=f"xqh{hh}", name=f"xqh{hh}")
                xqh.append(t)
                nc.sync.dma_start(
                    t[:],
                    _co_view(xq[:])[:, :, hh * (NQ // 2) : (hh + 1) * (NQ // 2)]
                    .bitcast(F32R),
                )

            # ---- weights (gamma-only scale: no stats dependency) ----
            # wq first on the scalar HWDGE ring: it gates the first K matmuls
            wq = wts.tile([P, CO, 3 * C], F32R)
            nc.scalar.dma_start(wq[:], _co_view(wqkvT[:]).bitcast(F32R))
            gam = small.tile([P, CO], F32)
            nc.scalar.dma_start(gam[:], gamma[:][0].rearrange("(co cp) -> cp co", cp=P))
            bet = small.tile([P, CO], F32)
            nc.scalar.dma_start(bet[:], beta[:][0].rearrange("(co cp) -> cp co", cp=P))
            wqs = wts.tile([P, CO, 3 * C], F32R)
            for co in range(CO):
                nc.vector.tensor_scalar_mul(
                    out=wqs[:, co, :],
                    in0=wq[:, co, :].bitcast(F32),
                    scalar1=gam[:, co : co + 1],
                )
            wpj = wts.tile([P, CO, C], F32R)
            nc.scalar.dma_start(wpj[:], _co_view(wprojT[:]).bitcast(F32R))
            qb_sb = small.tile([1, 3 * C], F32)
            nc.scalar.dma_start(qb_sb[:1, :], qkvb[:])
            pb_sb = small.tile([1, C], F32)
            nc.scalar.dma_start(pb_sb[:1, :], projb[:])

            # ---- groupnorm stats (DVE + GpSimd only; PE never stalls) ----
            mv = small.tile([P, 2], F32)
            nc.vector.bn_aggr(out=mv[:], in_=stt[:])
            st2 = small.tile([P, 2], F32)
            nc.vector.tensor_mul(out=st2[:, 1:2], in0=mv[:, 0:1], in1=mv[:, 0:1])
            nc.vector.tensor_add(out=st2[:, 1:2], in0=st2[:, 1:2], in1=mv[:, 1:2])
            nc.vector.tensor_copy(st2[:, 0:1], mv[:, 0:1])
            st_red = small.tile([P, 2], F32)
            nc.gpsimd.partition_all_reduce(
                st_red[:], st2[:], channels=P, reduce_op=bass_isa.ReduceOp.add
            )
            meanv = small.tile([P, 1], F32)
            nc.scalar.mul(out=meanv[:], in_=st_red[:, 0:1], mul=1.0 / P)
            sqm = small.tile([P, 1], F32)
            nc.scalar.mul(out=sqm[:], in_=st_red[:, 1:2], mul=1.0 / P)
            msq = small.tile([P, 1], F32)
            nc.vector.tensor_mul(out=msq[:], in0=meanv[:], in1=meanv[:])
            varv = small.tile([P, 1], F32)
            nc.vector.tensor_sub(out=varv[:], in0=sqm[:], in1=msq[:])
            epsb = small.tile([P, 1], F32)
            nc.vector.memset(epsb[:], EPS)
            stdv = small.tile([P, 1], F32)
            nc.scalar.activation(
                out=stdv[:], in_=varv[:], func=Act.Sqrt, bias=epsb[:], scale=1.0
            )
            rstd = small.tile([P, 1], F32)
            nc.vector.reciprocal(out=rstd[:], in_=stdv[:])
            rs2 = small.tile([P, 1], F32)  # rstd^2 (for the Q fold)
            nc.vector.tensor_mul(out=rs2[:], in0=rstd[:], in1=rstd[:])
            # A = gamma * rstd ; Bterm = beta - mean * A
            A = small.tile([P, CO], F32)
            nc.vector.tensor_scalar_mul(out=A[:], in0=gam[:], scalar1=rstd[:])
            Bt = small.tile([P, CO], F32)
            nc.vector.tensor_scalar_mul(out=Bt[:], in0=A[:], scalar1=meanv[:])
            nc.vector.tensor_sub(out=Bt[:], in0=bet[:], in1=Bt[:])
            Br = small.tile([P, CO], F32R)
            nc.vector.tensor_copy(Br[:], Bt[:])
            # proj weights absorb the V-path rstd factor
            wpjs = wts.tile([P, CO, C], F32R)
            for co in range(CO):
                nc.vector.tensor_scalar_mul(
                    out=wpjs[:, co, :], in0=wpj[:, co, :].bitcast(F32), scalar1=rstd[:]
                )
            ones32 = small.tile([P, P], F32)
            nc.vector.memset(ones32[:], 1.0)
            ones_r = small.tile([P, P], F32R)
            nc.vector.tensor_copy(ones_r[:], ones32[:])

            if _KNOB_STAGE < 2:
                return

            # ---------------- K = gamma-scaled Wk x (raw)  [c, j] ----------------
            # plain rounding copies: k-bias and rstd live on the Q side
            K_sb = big.tile([P, CO, HW], F32R)
            for d2 in range(CO):
                for jp in range(HW // 1024):
                    ps = mmp.tile([P, CHW0], F32, tag="qk")
                    for half in range(2):
                        jt = jp * 2 + half
                        for co in range(CO):
                            nc.tensor.matmul(
                                ps[:, half * 512 : (half + 1) * 512],
                                wqs[:, co, 256 + d2 * P : 256 + (d2 + 1) * P],
                                xps[jt // 2][
                                    :, co, (jt % 2) * 512 : (jt % 2 + 1) * 512
                                ],
                                start=(co == 0),
                                stop=(co == CO - 1),
                            )
                    nc.scalar.copy(
                        out=K_sb[:, d2, jp * 1024 : (jp + 1) * 1024], in_=ps[:]
                    )

            if _KNOB_STAGE < 3:
                return

            # ---------------- effective qkv bias (per-partition via bounce) ----
            ps_b1 = mmp.tile([P, CHW0], F32, tag="qk")
            ps_b2 = mmp.tile([P, CHW0], F32, tag="qk")
            for co in range(CO):
                st, sp = (co == 0), (co == CO - 1)
                nc.tensor.matmul(
                    ps_b1[:1, 0:512],
                    Br[:, co : co + 1],
                    wq[:, co, 0:512],
                    start=st,
                    stop=sp,
                )
                nc.tensor.matmul(
                    ps_b2[:1, 0:256],
                    Br[:, co : co + 1],
                    wq[:, co, 512:768],
                    start=st,
                    stop=sp,
                )
            bflat = small.tile([1, 3 * C], F32)
            nc.vector.tensor_add(
                out=bflat[:1, 0:512], in0=ps_b1[:1, 0:512], in1=qb_sb[:1, 0:512]
            )
            nc.vector.tensor_add(
                out=bflat[:1, 512:768], in0=ps_b2[:1, 0:256], in1=qb_sb[:1, 512:768]
            )
            bd = dramp.tile([1, 3 * C], F32)
            nc.scalar.dma_start(bd[:1, :], bflat[:1, :])
            bqk = small.tile([P, 3, CO], F32)
            nc.scalar.dma_start(
                bqk[:], bd[:][0].rearrange("(w co cp) -> cp w co", cp=P, w=3)
            )
            bqs = small.tile([P, CO], F32)  # rstd * bq, the Q-side additive
            nc.vector.tensor_scalar_mul(out=bqs[:], in0=bqk[:, 0, :], scalar1=rstd[:])
            bqs2 = bqs

            if _KNOB_STAGE < 4:
                return

            # ---------------- V^T = x^T (gamma-scaled Wv)  [j, c] ----------------
            NJB = HW // P  # 32 j-blocks
            VT_sb = big.tile([P, NJB, C], F32R)
            for jg in range(NJB // 4):
                ps = mmp.tile([P, CHW0], F32, tag="qk")
                for k4 in range(4):
                    jb = jg * 4 + k4
                    for co in range(CO):
                        nc.tensor.matmul(
                            ps[:, k4 * C : (k4 + 1) * C],
                            xps[jb // 8][:, co, (jb % 8) * P : (jb % 8 + 1) * P],
                            wqs[:, co, 512:768],
                            start=(co == 0),
                            stop=(co == CO - 1),
                        )
                if jg % 2 == 0:
                    nc.scalar.copy(
                        out=VT_sb[:, jg * 4 : (jg + 1) * 4, :], in_=ps[:]
                    )
                else:
                    nc.vector.tensor_copy(
                        VT_sb[:, jg * 4 : (jg + 1) * 4, :], ps[:]
                    )

            if _KNOB_STAGE < 5:
                return

            # ------------- Q~ = rstd^2 * (gamma-scaled Wq) xq + rstd*bq  [c, i] -----
            Q_sbs = [
                big.tile([P, CO, CHW0], F32R, tag=f"Qc{k}", name=f"Qc{k}")
                for k in range(NQ // CHW0)
            ]
            for ip in range(NQ // CHW0):
                for d2 in range(CO):
                    ps = mmp.tile([P, CHW0], F32, tag="qk")
                    for half in range(2):
                        for co in range(CO):
                            nc.tensor.matmul(
                                ps[:, half * 512 : (half + 1) * 512],
                                wqs[:, co, d2 * P : (d2 + 1) * P],
                                xqh[ip][:, co, half * 512 : (half + 1) * 512],
                                start=(co == 0),
                                stop=(co == CO - 1),
                            )
                    if d2 == 0:
                        nc.scalar.activation(
                            out=Q_sbs[ip][:, d2, :],
                            in_=ps[:],
                            func=Act.Identity,
                            bias=bqs2[:, d2 : d2 + 1],
                            scale=rs2[:],
                        )
                    else:
                        nc.vector.tensor_scalar(
                            out=Q_sbs[ip][:, d2, :],
                            in0=ps[:],
                            scalar1=rs2[:],
                            scalar2=bqs[:, d2 : d2 + 1],
                            op0=Alu.mult,
                            op1=Alu.add,
                        )

            # proj bias absorbs V bias: bpj = projb + Wp @ bv
            bqkv_r = small.tile([P, CO], F32R)
            nc.vector.tensor_copy(bqkv_r[:], bqk[:, 2, :])
            ps_pb = mmp.tile([P, CHW0], F32, tag="qk")
            for co in range(CO):
                nc.tensor.matmul(
                    ps_pb[:1, 0:C],
                    bqkv_r[:, co : co + 1],
                    wpj[:, co, :],
                    start=(co == 0),
                    stop=(co == CO - 1),
                )
            pb_flat = small.tile([1, C], F32)
            nc.vector.tensor_add(
                out=pb_flat[:1, :], in0=ps_pb[:1, 0:C], in1=pb_sb[:1, :]
            )
            bd2 = dramp.tile([1, C], F32)
            nc.scalar.dma_start(bd2[:1, :], pb_flat[:1, :])
            bpj = small.tile([P, CO], F32)
            nc.scalar.dma_start(bpj[:], bd2[:][0].rearrange("(co cp) -> cp co", cp=P))

            if _KNOB_STAGE < 6:
                return

            # ---------------- attention (streamed, transposed) ----------------
            # chunk widths [1024, 512, 512]: wide chunks amortize per-op
            # overheads; the narrow final chunks halve the exposed tail chain
            CHUNKS = [(0, 1024), (1024, 512), (1536, 512)]
            ao_sbs = [
                big.tile([P, CO, cw], F32R, tag=f"aoc{k}", name=f"aoc{k}")
                for k, (_, cw) in enumerate(CHUNKS)
            ]

            def _emit_proj(pci):
                coff, cw = CHUNKS[pci]
                for i2 in range(cw // 512):
                    for d2 in range(CO):
                        s = slice(coff + i2 * 512, coff + (i2 + 1) * 512)
                        pps = mmp.tile([P, CHW0], F32, tag="qk", name="pps")
                        for co in range(CO):
                            nc.tensor.matmul(
                                pps[:, 0:512],
                                wpjs[:, co, d2 * P : (d2 + 1) * P],
                                ao_sbs[pci][:, co, i2 * 512 : (i2 + 1) * 512],
                                start=(co == 0),
                                stop=(co == CO - 1),
                            )
                        ot = outp.tile([P, 512], F32, tag="ot", name="ot")
                        nc.scalar.activation(
                            out=ot[:],
                            in_=pps[:, 0:512],
                            func=Act.Identity,
                            bias=bpj[:, d2 : d2 + 1],
                            scale=1.0,
                        )
                        g0 = coff + i2 * 512
                        yt = outp.tile([P, 512], F32, tag="yt", name="yt")
                        nc.vector.tensor_add(
                            out=yt[:],
                            in0=ot[:],
                            in1=xqh[g0 // 1024][
                                :, d2, g0 % 1024 : g0 % 1024 + 512
                            ].bitcast(F32),
                        )
                        nc.sync.dma_start(_co_view(y[:])[:, d2, s], yt[:])

            for ci, (coff, cw) in enumerate(CHUNKS):
                ni2 = cw // 512
                qi, qoff = coff // CHW0, coff % CHW0
                # denominator partials: even j-blocks on DVE, odd on GpSimd
                dacc_v = dap.tile([P, CHW0], F32R, tag="dacc_v")
                nc.vector.memset(dacc_v[:, 0:cw].bitcast(F32), 0.0)
                dacc_g = dap.tile([P, CHW0], F32, tag="dacc_g")
                nc.gpsimd.memset(dacc_g[:, 0:cw], 0.0)
                pvs = [
                    pvp.tile([P, 512], F32, tag=f"pv{k}", name=f"pv{k}")
                    for k in range(CO * ni2)
                ]
                exs = [None] * NJB
                # software-pipelined 2-deep: QK/exp for jb before PV for jb-2
                for jb in range(NJB + 3):
                    if jb < NJB:
                        qk = mmp.tile([P, CHW0], F32, tag="qk")
                        for i2 in range(ni2):
                            for co in range(CO):
                                nc.tensor.matmul(
                                    qk[:, i2 * 512 : (i2 + 1) * 512],
                                    K_sb[:, co, jb * P : (jb + 1) * P],
                                    Q_sbs[qi][
                                        :, co, qoff + i2 * 512 : qoff + (i2 + 1) * 512
                                    ],
                                    start=(co == 0),
                                    stop=(co == CO - 1),
                                )
                        ex = expp.tile([P, CHW0], F32R, tag="ex")
                        nc.scalar.activation(
                            out=ex[:, 0:cw], in_=qk[:, 0:cw], func=Act.Exp, scale=SCALE
                        )
                        if jb % 2 == 0:
                            nc.vector.tensor_add(
                                out=dacc_v[:, 0:cw],
                                in0=dacc_v[:, 0:cw].bitcast(F32),
                                in1=ex[:, 0:cw].bitcast(F32),
                            )
                        else:
                            nc.gpsimd.tensor_add(
                                out=dacc_g[:, 0:cw],
                                in0=dacc_g[:, 0:cw],
                                in1=ex[:, 0:cw].bitcast(F32),
                            )
                        exs[jb] = ex
                    if jb >= 3:
                        pj = jb - 3
                        for c2 in range(CO):
                            for i2 in range(ni2):
                                nc.tensor.matmul(
                                    pvs[c2 * ni2 + i2][:],
                                    VT_sb[:, pj, c2 * P : (c2 + 1) * P],
                                    exs[pj][:, i2 * 512 : (i2 + 1) * 512],
                                    start=(pj == 0),
                                    stop=(pj == NJB - 1),
                                )
                    if jb == 3 and ci > 0:
                        _emit_proj(ci - 1)

                dgr = dap.tile([P, CHW0], F32R, tag="dscr")
                nc.vector.tensor_copy(dgr[:, 0:cw], dacc_g[:, 0:cw])
                dps = mmp.tile([P, CHW0], F32, tag="qk")
                for i2 in range(ni2):
                    s = slice(i2 * 512, (i2 + 1) * 512)
                    nc.tensor.matmul(
                        dps[:, s], ones_r[:], dacc_v[:, s], start=True, stop=False
                    )
                    nc.tensor.matmul(
                        dps[:, s], ones_r[:], dgr[:, s], start=False, stop=True
                    )
                rec = dap.tile([P, CHW0], F32, tag="dscr")
                for i2 in range(ni2):
                    si = slice(i2 * 512, (i2 + 1) * 512)
                    nc.vector.reciprocal(out=rec[:, si], in_=dps[:, si])
                    for c2 in range(CO):
                        nc.vector.tensor_mul(
                            out=ao_sbs[ci][:, c2, si],
                            in0=pvs[c2 * ni2 + i2][:],
                            in1=rec[:, si],
                        )
                if ci == len(CHUNKS) - 1:
                    _emit_proj(ci)



def _get_prog():
    global _prog
    if _prog is None:
        _prog = _build()
    return _prog


def kernel(x, norm_gamma, norm_beta, qkv_w, qkv_b, proj_w, proj_b, **_ignored):
    x = np.ascontiguousarray(np.asarray(x, dtype=np.float32))
    gamma = np.ascontiguousarray(np.asarray(norm_gamma, dtype=np.float32))
    beta = np.ascontiguousarray(np.asarray(norm_beta, dtype=np.float32))
    qkv_b = np.ascontiguousarray(np.asarray(qkv_b, dtype=np.float32))
    proj_b = np.ascontiguousarray(np.asarray(proj_b, dtype=np.float32))
    wqkvT = np.ascontiguousarray(np.asarray(qkv_w, dtype=np.float32).T)
    wprojT = np.ascontiguousarray(np.asarray(proj_w, dtype=np.float32).T)

    nc = _get_prog()
    in_maps = []
    for c in range(NCORES):
        b, h = divmod(c, 2)
        xb = np.ascontiguousarray(x[b].reshape(C, HW))
        xqh = np.ascontiguousarray(xb[:, h * NQ : (h + 1) * NQ])
        in_maps.append(
            dict(
                xr=xb,
                xq=xqh,
                wqkvT=wqkvT,
                wprojT=wprojT,
                qkvb=qkv_b.reshape(1, 3 * C),
                projb=proj_b.reshape(1, C),
                gamma=gamma.reshape(1, C),
                beta=beta.reshape(1, C),
            )
        )
    res = run_bass_kernel_spmd(nc, in_maps, core_ids=list(range(NCORES)))
    out = np.empty((B, C, HW), np.float32)
    for c in range(NCORES):
        b, h = divmod(c, 2)
        out[b, :, h * NQ : (h + 1) * NQ] = res.results[c]["y"]
    return out.reshape(B, C, 64, 64)


# revision 57
# speedup vs baseline: 1.0665x; 1.0173x over previous
"""AttentionBlock (GroupNorm + single-head 1x1-conv attention + residual) on 8 trn2 cores.

Sharding: 8 cores = (batch b in 0..3) x (query-half h in 0..1). Each core computes
the full attention rows for its 2048 query positions of its sample.

Per-core pipeline (all matmuls in float32r: full PE rate for moving-dim >= 256,
near-fp32 precision in practice):
  1. x loads in pieces; GroupNorm(1 group) stats via bn_stats overlapped with the
     DMA; the cross-partition combine runs on GpSimd (partition_all_reduce) so
     the PE instruction stream is never blocked by statistics.
  2. qkv weights are scaled by gamma only (known immediately), so all QKV
     matmuls start as soon as x pieces land. The scalar rstd and the effective
     q-bias are folded into Q alone: Q~ = rstd^2*q_raw + rstd*bq. Then
     logits = Q~^T K_raw reproduces rstd^2*q.k + u[j]; the k-bias and the
     constant terms cancel inside softmax. K and V therefore get plain
     (stats-free) PSUM->SBUF rounding copies. The V-path rstd is folded into
     the proj weights, and the V bias exactly into the proj bias (softmax rows
     sum to 1): out = (rstd*Wp) @ ao_raw + (Wp @ bv + pb).
  3. Q[c, i] (own half), K[c, j] (full), V^T[j, c] (full, produced transposed
     directly by swapping matmul operands).
  4. Streamed attention over 32 j-blocks x 1024-wide i-chunks: logits^T[jb, i]
     = K_blk^T Q~ into a 2-bank PSUM tile, one fused scale+exp on ScalarE per
     block, denominator partials alternating VectorE/GpSimd (no port contention
     at 1x), PV accumulated over j-blocks in PSUM. Softmax max-subtraction is
     skipped: logits are bounded (|l| <= 16) so exp is safe in fp32 and the
     result is mathematically identical.
  5. Denominator replicated across partitions with a ones-matmul, reciprocal,
     applied to PV. proj + bias + residual per chunk, overlapping the next
     chunk's attention.
"""

import os

import numpy as np

import concourse.bass as bass
import concourse.bass_isa as bass_isa
import concourse.mybir as mybir
import concourse.tile as tile
from concourse import bacc
from concourse.bass_utils import run_bass_kernel_spmd

# perf-experiment knobs (only read at build time; defaults = production)
_KNOB_NCH = int(os.environ.get("K_NCH", "2"))  # attention i-chunks emitted
_KNOB_MMBUFS = int(os.environ.get("K_MMBUFS", "3"))
_KNOB_EXBUFS = int(os.environ.get("K_EXBUFS", "4"))
_KNOB_STAGE = int(os.environ.get("K_STAGE", "9"))

B, C, HW = 4, 256, 4096
CHW0 = 1024  # attention i-chunk width == qk psum tile width (2 banks)
P, CO = 128, 2
NQ = HW // 2  # queries per core
NCORES = 8
SCALE = float(C) ** -0.5
EPS = 1e-5
F32, F32R = mybir.dt.float32, mybir.dt.float32r
Act = mybir.ActivationFunctionType
Alu = mybir.AluOpType

_prog = None


def _co_view(ap):
    """[C, N] dram AP -> [128, 2, N] (partition cp, free (co, n)); c = co*128+cp."""
    return ap.rearrange("(co cp) n -> cp co n", cp=P)


def _build():
    nc = bacc.Bacc(None, target_bir_lowering=False)
    _build_body(nc)
    nc.compile()
    return nc


def _build_body(nc):
    xr = nc.dram_tensor("xr", [C, HW], F32, kind="ExternalInput")
    xq = nc.dram_tensor("xq", [C, NQ], F32, kind="ExternalInput")
    wqkvT = nc.dram_tensor("wqkvT", [C, 3 * C], F32, kind="ExternalInput")
    wprojT = nc.dram_tensor("wprojT", [C, C], F32, kind="ExternalInput")
    qkvb = nc.dram_tensor("qkvb", [1, 3 * C], F32, kind="ExternalInput")
    projb = nc.dram_tensor("projb", [1, C], F32, kind="ExternalInput")
    gamma = nc.dram_tensor("gamma", [1, C], F32, kind="ExternalInput")
    beta = nc.dram_tensor("beta", [1, C], F32, kind="ExternalInput")
    y = nc.dram_tensor("y", [C, NQ], F32, kind="ExternalOutput")

    with tile.TileContext(nc) as tc:
        with (
            tc.tile_pool(name="big", bufs=1) as big,
            tc.tile_pool(name="wts", bufs=1) as wts,
            tc.tile_pool(name="small", bufs=1) as small,
            tc.tile_pool(name="expp", bufs=_KNOB_EXBUFS) as expp,
            tc.tile_pool(name="dap", bufs=1) as dap,
            tc.tile_pool(name="outp", bufs=2) as outp,
            tc.tile_pool(name="mm", bufs=_KNOB_MMBUFS, space="PSUM") as mmp,
            tc.tile_pool(name="pv", bufs=1, space="PSUM") as pvp,
            tc.tile_pool(name="dram", bufs=1, space="DRAM") as dramp,
        ):
            # ---- x load as 4 separate piece tiles (whole-tile dep tracking:
            # separate tiles let bn/K/VT start as each piece lands) ----
            NXP = 4
            XPW = HW // NXP  # 1024
            xps = []
            stt = small.tile([P, 16, 6], F32)
            for p_ in range(NXP):
                nsl = slice(p_ * XPW, (p_ + 1) * XPW)
                xp = big.tile([P, CO, XPW], F32R, tag=f"xp{p_}", name=f"xp{p_}")
                xps.append(xp)
                nc.sync.dma_start(xp[:], _co_view(xr[:])[:, :, nsl].bitcast(F32R))
                for co in range(CO):
                    for s2 in range(XPW // 512):
                        nc.vector.bn_stats(
                            out=stt[:, p_ * 4 + co * 2 + s2, :],
                            in_=xp[:, co, s2 * 512 : (s2 + 1) * 512].bitcast(F32),
                        )
            # xq rides the same (sync) HWDGE ring so it starts after the
            # stats-critical xr pieces have drained; split in half so chunk 0's
            # Q matmuls start a transfer earlier
            xqh = []
            for hh in range(2):
                t = big.tile([P, CO, NQ // 2], F32R, tag=f"xqh{hh}", name=f"xqh{hh}")
                xqh.append(t)
                nc.sync.dma_start(
                    t[:],
                    _co_view(xq[:])[:, :, hh * (NQ // 2) : (hh + 1) * (NQ // 2)]
                    .bitcast(F32R),
                )

            # ---- weights (gamma-only scale: no stats dependency) ----
            # wq first on the scalar HWDGE ring: it gates the first K matmuls
            wq = wts.tile([P, CO, 3 * C], F32R)
            nc.scalar.dma_start(wq[:], _co_view(wqkvT[:]).bitcast(F32R))
            gam = small.tile([P, CO], F32)
            nc.scalar.dma_start(gam[:], gamma[:][0].rearrange("(co cp) -> cp co", cp=P))
            bet = small.tile([P, CO], F32)
            nc.scalar.dma_start(bet[:], beta[:][0].rearrange("(co cp) -> cp co", cp=P))
            wqs = wts.tile([P, CO, 3 * C], F32R)
            for co in range(CO):
                nc.vector.tensor_scalar_mul(
                    out=wqs[:, co, :],
                    in0=wq[:, co, :].bitcast(F32),
                    scalar1=gam[:, co : co + 1],
                )
            wpj = wts.tile([P, CO, C], F32R)
            nc.scalar.dma_start(wpj[:], _co_view(wprojT[:]).bitcast(F32R))
            qb_sb = small.tile([1, 3 * C], F32)
            nc.scalar.dma_start(qb_sb[:1, :], qkvb[:])
            pb_sb = small.tile([1, C], F32)
            nc.scalar.dma_start(pb_sb[:1, :], projb[:])

            # ---- groupnorm stats (DVE + GpSimd only; PE never stalls) ----
            mv = small.tile([P, 2], F32)
            nc.vector.bn_aggr(out=mv[:], in_=stt[:])
            st2 = small.tile([P, 2], F32)
            nc.vector.tensor_mul(out=st2[:, 1:2], in0=mv[:, 0:1], in1=mv[:, 0:1])
            nc.vector.tensor_add(out=st2[:, 1:2], in0=st2[:, 1:2], in1=mv[:, 1:2])
            nc.vector.tensor_copy(st2[:, 0:1], mv[:, 0:1])
            st_red = small.tile([P, 2], F32)
            nc.gpsimd.partition_all_reduce(
                st_red[:], st2[:], channels=P, reduce_op=bass_isa.ReduceOp.add
            )
            meanv = small.tile([P, 1], F32)
            nc.scalar.mul(out=meanv[:], in_=st_red[:, 0:1], mul=1.0 / P)
            sqm = small.tile([P, 1], F32)
            nc.scalar.mul(out=sqm[:], in_=st_red[:, 1:2], mul=1.0 / P)
            msq = small.tile([P, 1], F32)
            nc.vector.tensor_mul(out=msq[:], in0=meanv[:], in1=meanv[:])
            varv = small.tile([P, 1], F32)
            nc.vector.tensor_sub(out=varv[:], in0=sqm[:], in1=msq[:])
            epsb = small.tile([P, 1], F32)
            nc.vector.memset(epsb[:], EPS)
            stdv = small.tile([P, 1], F32)
            nc.scalar.activation(
                out=stdv[:], in_=varv[:], func=Act.Sqrt, bias=epsb[:], scale=1.0
            )
            rstd = small.tile([P, 1], F32)
            nc.vector.reciprocal(out=rstd[:], in_=stdv[:])
            rs2 = small.tile([P, 1], F32)  # rstd^2 (for the Q fold)
            nc.vector.tensor_mul(out=rs2[:], in0=rstd[:], in1=rstd[:])
            # A = gamma * rstd ; Bterm = beta - mean * A
            A = small.tile([P, CO], F32)
            nc.vector.tensor_scalar_mul(out=A[:], in0=gam[:], scalar1=rstd[:])
            Bt = small.tile([P, CO], F32)
            nc.vector.tensor_scalar_mul(out=Bt[:], in0=A[:], scalar1=meanv[:])
            nc.vector.tensor_sub(out=Bt[:], in0=bet[:], in1=Bt[:])
            Br = small.tile([P, CO], F32R)
            nc.vector.tensor_copy(Br[:], Bt[:])
            # proj weights absorb the V-path rstd factor
            wpjs = wts.tile([P, CO, C], F32R)
            for co in range(CO):
                nc.vector.tensor_scalar_mul(
                    out=wpjs[:, co, :], in0=wpj[:, co, :].bitcast(F32), scalar1=rstd[:]
                )
            ones32 = small.tile([P, P], F32)
            nc.vector.memset(ones32[:], 1.0)
            ones_r = small.tile([P, P], F32R)
            nc.vector.tensor_copy(ones_r[:], ones32[:])

            if _KNOB_STAGE < 2:
                return

            # ---------------- K = gamma-scaled Wk x (raw)  [c, j] ----------------
            # plain rounding copies: k-bias and rstd live on the Q side
            K_sb = big.tile([P, CO, HW], F32R)
            for d2 in range(CO):
                for jp in range(HW // 1024):
                    ps = mmp.tile([P, CHW0], F32, tag="qk")
                    for half in range(2):
                        jt = jp * 2 + half
                        for co in range(CO):
                            nc.tensor.matmul(
                                ps[:, half * 512 : (half + 1) * 512],
                                wqs[:, co, 256 + d2 * P : 256 + (d2 + 1) * P],
                                xps[jt // 2][
                                    :, co, (jt % 2) * 512 : (jt % 2 + 1) * 512
                                ],
                                start=(co == 0),
                                stop=(co == CO - 1),
                            )
                    nc.scalar.copy(
                        out=K_sb[:, d2, jp * 1024 : (jp + 1) * 1024], in_=ps[:]
                    )

            if _KNOB_STAGE < 3:
                return

            # ---------------- effective qkv bias (per-partition via bounce) ----
            ps_b1 = mmp.tile([P, CHW0], F32, tag="qk")
            ps_b2 = mmp.tile([P, CHW0], F32, tag="qk")
            for co in range(CO):
                st, sp = (co == 0), (co == CO - 1)
                nc.tensor.matmul(
                    ps_b1[:1, 0:512],
                    Br[:, co : co + 1],
                    wq[:, co, 0:512],
                    start=st,
                    stop=sp,
                )
                nc.tensor.matmul(
                    ps_b2[:1, 0:256],
                    Br[:, co : co + 1],
                    wq[:, co, 512:768],
                    start=st,
                    stop=sp,
                )
            bflat = small.tile([1, 3 * C], F32)
            nc.vector.tensor_add(
                out=bflat[:1, 0:512], in0=ps_b1[:1, 0:512], in1=qb_sb[:1, 0:512]
            )
            nc.vector.tensor_add(
                out=bflat[:1, 512:768], in0=ps_b2[:1, 0:256], in1=qb_sb[:1, 512:768]
            )
            bd = dramp.tile([1, 3 * C], F32)
            nc.scalar.dma_start(bd[:1, :], bflat[:1, :])
            bqk = small.tile([P, 3, CO], F32)
            nc.scalar.dma_start(
                bqk[:], bd[:][0].rearrange("(w co cp) -> cp w co", cp=P, w=3)
            )
            bqs = small.tile([P, CO], F32)  # rstd * bq, the Q-side additive
            nc.vector.tensor_scalar_mul(out=bqs[:], in0=bqk[:, 0, :], scalar1=rstd[:])
            bqs2 = bqs

            if _KNOB_STAGE < 4:
                return

            # ---------------- V^T = x^T (gamma-scaled Wv)  [j, c] ----------------
            NJB = HW // P  # 32 j-blocks
            VT_sb = big.tile([P, NJB, C], F32R)
            for jg in range(NJB // 4):
                ps = mmp.tile([P, CHW0], F32, tag="qk")
                for k4 in range(4):
                    jb = jg * 4 + k4
                    for co in range(CO):
                        nc.tensor.matmul(
                            ps[:, k4 * C : (k4 + 1) * C],
                            xps[jb // 8][:, co, (jb % 8) * P : (jb % 8 + 1) * P],
                            wqs[:, co, 512:768],
                            start=(co == 0),
                            stop=(co == CO - 1),
                        )
                if jg % 2 == 0:
                    nc.scalar.copy(
                        out=VT_sb[:, jg * 4 : (jg + 1) * 4, :], in_=ps[:]
                    )
                else:
                    nc.vector.tensor_copy(
                        VT_sb[:, jg * 4 : (jg + 1) * 4, :], ps[:]
                    )

            if _KNOB_STAGE < 5:
                return

            # ------------- Q~ = rstd^2 * (gamma-scaled Wq) xq + rstd*bq  [c, i] -----
            Q_sbs = [
                big.tile([P, CO, CHW0], F32R, tag=f"Qc{k}", name=f"Qc{k}")
                for k in range(NQ // CHW0)
            ]
            for ip in range(NQ // CHW0):
                for d2 in range(CO):
                    ps = mmp.tile([P, CHW0], F32, tag="qk")
                    for half in range(2):
                        for co in range(CO):
                            nc.tensor.matmul(
                                ps[:, half * 512 : (half + 1) * 512],
                                wqs[:, co, d2 * P : (d2 + 1) * P],
                                xqh[ip][:, co, half * 512 : (half + 1) * 512],
                                start=(co == 0),
                                stop=(co == CO - 1),
                            )
                    if d2 == 0:
                        nc.scalar.activation(
                            out=Q_sbs[ip][:, d2, :],
                            in_=ps[:],
                            func=Act.Identity,
                            bias=bqs2[:, d2 : d2 + 1],
                            scale=rs2[:],
                        )
                    else:
                        nc.vector.tensor_scalar(
                            out=Q_sbs[ip][:, d2, :],
                            in0=ps[:],
                            scalar1=rs2[:],
                            scalar2=bqs[:, d2 : d2 + 1],
                            op0=Alu.mult,
                            op1=Alu.add,
                        )

            # proj bias absorbs V bias: bpj = projb + Wp @ bv
            bqkv_r = small.tile([P, CO], F32R)
            nc.vector.tensor_copy(bqkv_r[:], bqk[:, 2, :])
            ps_pb = mmp.tile([P, CHW0], F32, tag="qk")
            for co in range(CO):
                nc.tensor.matmul(
                    ps_pb[:1, 0:C],
                    bqkv_r[:, co : co + 1],
                    wpj[:, co, :],
                    start=(co == 0),
                    stop=(co == CO - 1),
                )
            pb_flat = small.tile([1, C], F32)
            nc.vector.tensor_add(
                out=pb_flat[:1, :], in0=ps_pb[:1, 0:C], in1=pb_sb[:1, :]
            )
            bd2 = dramp.tile([1, C], F32)
            nc.scalar.dma_start(bd2[:1, :], pb_flat[:1, :])
            bpj = small.tile([P, CO], F32)
            nc.scalar.dma_start(bpj[:], bd2[:][0].rearrange("(co cp) -> cp co", cp=P))

            if _KNOB_STAGE < 6:
                return

            # ---------------- attention (streamed, transposed) ----------------
            # chunk widths [1024, 512, 512]: wide chunks amortize per-op
            # overheads; the narrow final chunks halve the exposed tail chain
            CHUNKS = [(0, 512), (512, 512), (1024, 512), (1536, 512)]
            ao_sbs = [
                big.tile([P, CO, cw], F32R, tag=f"aoc{k}", name=f"aoc{k}")
                for k, (_, cw) in enumerate(CHUNKS)
            ]

            def _emit_proj(pci):
                coff, cw = CHUNKS[pci]
                for i2 in range(cw // 512):
                    for d2 in range(CO):
                        s = slice(coff + i2 * 512, coff + (i2 + 1) * 512)
                        pps = mmp.tile([P, CHW0], F32, tag="qk", name="pps")
                        for co in range(CO):
                            nc.tensor.matmul(
                                pps[:, 0:512],
                                wpjs[:, co, d2 * P : (d2 + 1) * P],
                                ao_sbs[pci][:, co, i2 * 512 : (i2 + 1) * 512],
                                start=(co == 0),
                                stop=(co == CO - 1),
                            )
                        ot = outp.tile([P, 512], F32, tag="ot", name="ot")
                        nc.scalar.activation(
                            out=ot[:],
                            in_=pps[:, 0:512],
                            func=Act.Identity,
                            bias=bpj[:, d2 : d2 + 1],
                            scale=1.0,
                        )
                        g0 = coff + i2 * 512
                        yt = outp.tile([P, 512], F32, tag="yt", name="yt")
                        nc.vector.tensor_add(
                            out=yt[:],
                            in0=ot[:],
                            in1=xqh[g0 // 1024][
                                :, d2, g0 % 1024 : g0 % 1024 + 512
                            ].bitcast(F32),
                        )
                        nc.sync.dma_start(_co_view(y[:])[:, d2, s], yt[:])

            for ci, (coff, cw) in enumerate(CHUNKS):
                ni2 = cw // 512
                qi, qoff = coff // CHW0, coff % CHW0
                # denominator partials: even j-blocks on DVE, odd on GpSimd
                dacc_v = dap.tile([P, CHW0], F32R, tag="dacc_v")
                nc.vector.memset(dacc_v[:, 0:cw].bitcast(F32), 0.0)
                dacc_g = dap.tile([P, CHW0], F32, tag="dacc_g")
                nc.gpsimd.memset(dacc_g[:, 0:cw], 0.0)
                pvs = [
                    pvp.tile([P, 512], F32, tag=f"pv{k}", name=f"pv{k}")
                    for k in range(CO * ni2)
                ]
                exs = [None] * NJB
                # software-pipelined 2-deep: QK/exp for jb before PV for jb-2
                for jb in range(NJB + 3):
                    if jb < NJB:
                        qk = mmp.tile([P, CHW0], F32, tag="qk")
                        for i2 in range(ni2):
                            for co in range(CO):
                                nc.tensor.matmul(
                                    qk[:, i2 * 512 : (i2 + 1) * 512],
                                    K_sb[:, co, jb * P : (jb + 1) * P],
                                    Q_sbs[qi][
                                        :, co, qoff + i2 * 512 : qoff + (i2 + 1) * 512
                                    ],
                                    start=(co == 0),
                                    stop=(co == CO - 1),
                                )
                        ex = expp.tile([P, CHW0], F32R, tag="ex")
                        nc.scalar.activation(
                            out=ex[:, 0:cw], in_=qk[:, 0:cw], func=Act.Exp, scale=SCALE
                        )
                        if jb % 2 == 0:
                            nc.vector.tensor_add(
                                out=dacc_v[:, 0:cw],
                                in0=dacc_v[:, 0:cw].bitcast(F32),
                                in1=ex[:, 0:cw].bitcast(F32),
                            )
                        else:
                            nc.gpsimd.tensor_add(
                                out=dacc_g[:, 0:cw],
                                in0=dacc_g[:, 0:cw],
                                in1=ex[:, 0:cw].bitcast(F32),
                            )
                        exs[jb] = ex
                    if jb >= 3:
                        pj = jb - 3
                        for c2 in range(CO):
                            for i2 in range(ni2):
                                nc.tensor.matmul(
                                    pvs[c2 * ni2 + i2][:],
                                    VT_sb[:, pj, c2 * P : (c2 + 1) * P],
                                    exs[pj][:, i2 * 512 : (i2 + 1) * 512],
                                    start=(pj == 0),
                                    stop=(pj == NJB - 1),
                                )
                    if jb == 8 and ci > 0:
                        _emit_proj(ci - 1)

                dgr = dap.tile([P, CHW0], F32R, tag="dscr")
                nc.vector.tensor_copy(dgr[:, 0:cw], dacc_g[:, 0:cw])
                dps = mmp.tile([P, CHW0], F32, tag="qk")
                for i2 in range(ni2):
                    s = slice(i2 * 512, (i2 + 1) * 512)
                    nc.tensor.matmul(
                        dps[:, s], ones_r[:], dacc_v[:, s], start=True, stop=False
                    )
                    nc.tensor.matmul(
                        dps[:, s], ones_r[:], dgr[:, s], start=False, stop=True
                    )
                rec = dap.tile([P, CHW0], F32, tag="dscr")
                for i2 in range(ni2):
                    si = slice(i2 * 512, (i2 + 1) * 512)
                    nc.vector.reciprocal(out=rec[:, si], in_=dps[:, si])
                    for c2 in range(CO):
                        nc.vector.tensor_mul(
                            out=ao_sbs[ci][:, c2, si],
                            in0=pvs[c2 * ni2 + i2][:],
                            in1=rec[:, si],
                        )
                if ci == len(CHUNKS) - 1:
                    _emit_proj(ci)



def _get_prog():
    global _prog
    if _prog is None:
        _prog = _build()
    return _prog


def kernel(x, norm_gamma, norm_beta, qkv_w, qkv_b, proj_w, proj_b, **_ignored):
    x = np.ascontiguousarray(np.asarray(x, dtype=np.float32))
    gamma = np.ascontiguousarray(np.asarray(norm_gamma, dtype=np.float32))
    beta = np.ascontiguousarray(np.asarray(norm_beta, dtype=np.float32))
    qkv_b = np.ascontiguousarray(np.asarray(qkv_b, dtype=np.float32))
    proj_b = np.ascontiguousarray(np.asarray(proj_b, dtype=np.float32))
    wqkvT = np.ascontiguousarray(np.asarray(qkv_w, dtype=np.float32).T)
    wprojT = np.ascontiguousarray(np.asarray(proj_w, dtype=np.float32).T)

    nc = _get_prog()
    in_maps = []
    for c in range(NCORES):
        b, h = divmod(c, 2)
        xb = np.ascontiguousarray(x[b].reshape(C, HW))
        xqh = np.ascontiguousarray(xb[:, h * NQ : (h + 1) * NQ])
        in_maps.append(
            dict(
                xr=xb,
                xq=xqh,
                wqkvT=wqkvT,
                wprojT=wprojT,
                qkvb=qkv_b.reshape(1, 3 * C),
                projb=proj_b.reshape(1, C),
                gamma=gamma.reshape(1, C),
                beta=beta.reshape(1, C),
            )
        )
    res = run_bass_kernel_spmd(nc, in_maps, core_ids=list(range(NCORES)))
    out = np.empty((B, C, HW), np.float32)
    for c in range(NCORES):
        b, h = divmod(c, 2)
        out[b, :, h * NQ : (h + 1) * NQ] = res.results[c]["y"]
    return out.reshape(B, C, 64, 64)


# revision 59
# speedup vs baseline: 1.0939x; 1.0257x over previous
"""AttentionBlock (GroupNorm + single-head 1x1-conv attention + residual) on 8 trn2 cores.

Sharding: 8 cores = (batch b in 0..3) x (query-half h in 0..1). Each core computes
the full attention rows for its 2048 query positions of its sample.

Per-core pipeline (all matmuls in float32r: full PE rate for moving-dim >= 256,
near-fp32 precision in practice):
  1. x loads in pieces; GroupNorm(1 group) stats via bn_stats overlapped with the
     DMA; the cross-partition combine runs on GpSimd (partition_all_reduce) so
     the PE instruction stream is never blocked by statistics.
  2. qkv weights are scaled by gamma only (known immediately), so all QKV
     matmuls start as soon as x pieces land. The scalar rstd and the effective
     q-bias are folded into Q alone: Q~ = rstd^2*q_raw + rstd*bq. Then
     logits = Q~^T K_raw reproduces rstd^2*q.k + u[j]; the k-bias and the
     constant terms cancel inside softmax. K and V therefore get plain
     (stats-free) PSUM->SBUF rounding copies. The V-path rstd is folded into
     the proj weights, and the V bias exactly into the proj bias (softmax rows
     sum to 1): out = (rstd*Wp) @ ao_raw + (Wp @ bv + pb).
  3. Q[c, i] (own half), K[c, j] (full), V^T[j, c] (full, produced transposed
     directly by swapping matmul operands).
  4. Streamed attention over 32 j-blocks x 1024-wide i-chunks: logits^T[jb, i]
     = K_blk^T Q~ into a 2-bank PSUM tile, one fused scale+exp on ScalarE per
     block, denominator partials alternating VectorE/GpSimd (no port contention
     at 1x), PV accumulated over j-blocks in PSUM. Softmax max-subtraction is
     skipped: logits are bounded (|l| <= 16) so exp is safe in fp32 and the
     result is mathematically identical.
  5. Denominator replicated across partitions with a ones-matmul, reciprocal,
     applied to PV. proj + bias + residual per chunk, overlapping the next
     chunk's attention.
"""

import os

import numpy as np

import concourse.bass as bass
import concourse.bass_isa as bass_isa
import concourse.mybir as mybir
import concourse.tile as tile
from concourse import bacc
from concourse.bass_utils import run_bass_kernel_spmd

# perf-experiment knobs (only read at build time; defaults = production)
_KNOB_NCH = int(os.environ.get("K_NCH", "2"))  # attention i-chunks emitted
_KNOB_MMBUFS = int(os.environ.get("K_MMBUFS", "3"))
_KNOB_EXBUFS = int(os.environ.get("K_EXBUFS", "7"))
_KNOB_STAGE = int(os.environ.get("K_STAGE", "9"))

B, C, HW = 4, 256, 4096
CHW0 = 1024  # attention i-chunk width == qk psum tile width (2 banks)
P, CO = 128, 2
NQ = HW // 2  # queries per core
NCORES = 8
SCALE = float(C) ** -0.5
EPS = 1e-5
F32, F32R = mybir.dt.float32, mybir.dt.float32r
Act = mybir.ActivationFunctionType
Alu = mybir.AluOpType

_prog = None


def _co_view(ap):
    """[C, N] dram AP -> [128, 2, N] (partition cp, free (co, n)); c = co*128+cp."""
    return ap.rearrange("(co cp) n -> cp co n", cp=P)


def _build():
    nc = bacc.Bacc(None, target_bir_lowering=False)
    _build_body(nc)
    nc.compile()
    return nc


def _build_body(nc):
    xr = nc.dram_tensor("xr", [C, HW], F32, kind="ExternalInput")
    xq = nc.dram_tensor("xq", [C, NQ], F32, kind="ExternalInput")
    wqkvT = nc.dram_tensor("wqkvT", [C, 3 * C], F32, kind="ExternalInput")
    wprojT = nc.dram_tensor("wprojT", [C, C], F32, kind="ExternalInput")
    qkvb = nc.dram_tensor("qkvb", [1, 3 * C], F32, kind="ExternalInput")
    projb = nc.dram_tensor("projb", [1, C], F32, kind="ExternalInput")
    gamma = nc.dram_tensor("gamma", [1, C], F32, kind="ExternalInput")
    beta = nc.dram_tensor("beta", [1, C], F32, kind="ExternalInput")
    y = nc.dram_tensor("y", [C, NQ], F32, kind="ExternalOutput")

    with tile.TileContext(nc) as tc:
        with (
            tc.tile_pool(name="big", bufs=1) as big,
            tc.tile_pool(name="wts", bufs=1) as wts,
            tc.tile_pool(name="small", bufs=1) as small,
            tc.tile_pool(name="expp", bufs=_KNOB_EXBUFS) as expp,
            tc.tile_pool(name="dap", bufs=1) as dap,
            tc.tile_pool(name="outp", bufs=2) as outp,
            tc.tile_pool(name="mm", bufs=_KNOB_MMBUFS, space="PSUM") as mmp,
            tc.tile_pool(name="pv", bufs=1, space="PSUM") as pvp,
            tc.tile_pool(name="dram", bufs=1, space="DRAM") as dramp,
        ):
            # ---- x load as 4 separate piece tiles (whole-tile dep tracking:
            # separate tiles let bn/K/VT start as each piece lands) ----
            NXP = 4
            XPW = HW // NXP  # 1024
            xps = []
            stt = small.tile([P, 16, 6], F32)
            for p_ in range(NXP):
                nsl = slice(p_ * XPW, (p_ + 1) * XPW)
                xp = big.tile([P, CO, XPW], F32R, tag=f"xp{p_}", name=f"xp{p_}")
                xps.append(xp)
                nc.sync.dma_start(xp[:], _co_view(xr[:])[:, :, nsl].bitcast(F32R))
                for co in range(CO):
                    for s2 in range(XPW // 512):
                        nc.vector.bn_stats(
                            out=stt[:, p_ * 4 + co * 2 + s2, :],
                            in_=xp[:, co, s2 * 512 : (s2 + 1) * 512].bitcast(F32),
                        )
            # xq rides the same (sync) HWDGE ring so it starts after the
            # stats-critical xr pieces have drained; split in half so chunk 0's
            # Q matmuls start a transfer earlier
            xqh = []
            for hh in range(2):
                t = big.tile([P, CO, NQ // 2], F32R, tag=f"xqh{hh}", name=f"xqh{hh}")
                xqh.append(t)
                nc.sync.dma_start(
                    t[:],
                    _co_view(xq[:])[:, :, hh * (NQ // 2) : (hh + 1) * (NQ // 2)]
                    .bitcast(F32R),
                )

            # ---- weights (gamma-only scale: no stats dependency) ----
            # wq first on the scalar HWDGE ring: it gates the first K matmuls
            wq = wts.tile([P, CO, 3 * C], F32R)
            nc.sync.dma_start(wq[:], _co_view(wqkvT[:]).bitcast(F32R))
            gam = small.tile([P, CO], F32)
            nc.scalar.dma_start(gam[:], gamma[:][0].rearrange("(co cp) -> cp co", cp=P))
            bet = small.tile([P, CO], F32)
            nc.scalar.dma_start(bet[:], beta[:][0].rearrange("(co cp) -> cp co", cp=P))
            wqs = wts.tile([P, CO, 3 * C], F32R)
            for co in range(CO):
                nc.vector.tensor_scalar_mul(
                    out=wqs[:, co, :],
                    in0=wq[:, co, :].bitcast(F32),
                    scalar1=gam[:, co : co + 1],
                )
            wpj = wts.tile([P, CO, C], F32R)
            nc.scalar.dma_start(wpj[:], _co_view(wprojT[:]).bitcast(F32R))
            qb_sb = small.tile([1, 3 * C], F32)
            nc.scalar.dma_start(qb_sb[:1, :], qkvb[:])
            pb_sb = small.tile([1, C], F32)
            nc.scalar.dma_start(pb_sb[:1, :], projb[:])

            # ---- groupnorm stats (DVE + GpSimd only; PE never stalls) ----
            mv = small.tile([P, 2], F32)
            nc.vector.bn_aggr(out=mv[:], in_=stt[:])
            st2 = small.tile([P, 2], F32)
            nc.vector.tensor_mul(out=st2[:, 1:2], in0=mv[:, 0:1], in1=mv[:, 0:1])
            nc.vector.tensor_add(out=st2[:, 1:2], in0=st2[:, 1:2], in1=mv[:, 1:2])
            nc.vector.tensor_copy(st2[:, 0:1], mv[:, 0:1])
            st_red = small.tile([P, 2], F32)
            nc.gpsimd.partition_all_reduce(
                st_red[:], st2[:], channels=P, reduce_op=bass_isa.ReduceOp.add
            )
            meanv = small.tile([P, 1], F32)
            nc.scalar.mul(out=meanv[:], in_=st_red[:, 0:1], mul=1.0 / P)
            sqm = small.tile([P, 1], F32)
            nc.scalar.mul(out=sqm[:], in_=st_red[:, 1:2], mul=1.0 / P)
            msq = small.tile([P, 1], F32)
            nc.vector.tensor_mul(out=msq[:], in0=meanv[:], in1=meanv[:])
            varv = small.tile([P, 1], F32)
            nc.vector.tensor_sub(out=varv[:], in0=sqm[:], in1=msq[:])
            epsb = small.tile([P, 1], F32)
            nc.vector.memset(epsb[:], EPS)
            stdv = small.tile([P, 1], F32)
            nc.scalar.activation(
                out=stdv[:], in_=varv[:], func=Act.Sqrt, bias=epsb[:], scale=1.0
            )
            rstd = small.tile([P, 1], F32)
            nc.vector.reciprocal(out=rstd[:], in_=stdv[:])
            rs2 = small.tile([P, 1], F32)  # rstd^2 (for the Q fold)
            nc.vector.tensor_mul(out=rs2[:], in0=rstd[:], in1=rstd[:])
            # A = gamma * rstd ; Bterm = beta - mean * A
            A = small.tile([P, CO], F32)
            nc.vector.tensor_scalar_mul(out=A[:], in0=gam[:], scalar1=rstd[:])
            Bt = small.tile([P, CO], F32)
            nc.vector.tensor_scalar_mul(out=Bt[:], in0=A[:], scalar1=meanv[:])
            nc.vector.tensor_sub(out=Bt[:], in0=bet[:], in1=Bt[:])
            Br = small.tile([P, CO], F32R)
            nc.vector.tensor_copy(Br[:], Bt[:])
            # proj weights absorb the V-path rstd factor
            wpjs = wts.tile([P, CO, C], F32R)
            for co in range(CO):
                nc.vector.tensor_scalar_mul(
                    out=wpjs[:, co, :], in0=wpj[:, co, :].bitcast(F32), scalar1=rstd[:]
                )
            ones32 = small.tile([P, P], F32)
            nc.vector.memset(ones32[:], 1.0)
            ones_r = small.tile([P, P], F32R)
            nc.vector.tensor_copy(ones_r[:], ones32[:])

            if _KNOB_STAGE < 2:
                return

            # ---------------- K = gamma-scaled Wk x (raw)  [c, j] ----------------
            # plain rounding copies: k-bias and rstd live on the Q side
            K_sb = big.tile([P, CO, HW], F32R)
            for d2 in range(CO):
                for jp in range(HW // 1024):
                    ps = mmp.tile([P, CHW0], F32, tag="qk")
                    for half in range(2):
                        jt = jp * 2 + half
                        for co in range(CO):
                            nc.tensor.matmul(
                                ps[:, half * 512 : (half + 1) * 512],
                                wqs[:, co, 256 + d2 * P : 256 + (d2 + 1) * P],
                                xps[jt // 2][
                                    :, co, (jt % 2) * 512 : (jt % 2 + 1) * 512
                                ],
                                start=(co == 0),
                                stop=(co == CO - 1),
                            )
                    nc.scalar.copy(
                        out=K_sb[:, d2, jp * 1024 : (jp + 1) * 1024], in_=ps[:]
                    )

            if _KNOB_STAGE < 3:
                return

            # ---------------- effective qkv bias (per-partition via bounce) ----
            ps_b1 = mmp.tile([P, CHW0], F32, tag="qk")
            ps_b2 = mmp.tile([P, CHW0], F32, tag="qk")
            for co in range(CO):
                st, sp = (co == 0), (co == CO - 1)
                nc.tensor.matmul(
                    ps_b1[:1, 0:512],
                    Br[:, co : co + 1],
                    wq[:, co, 0:512],
                    start=st,
                    stop=sp,
                )
                nc.tensor.matmul(
                    ps_b2[:1, 0:256],
                    Br[:, co : co + 1],
                    wq[:, co, 512:768],
                    start=st,
                    stop=sp,
                )
            bflat = small.tile([1, 3 * C], F32)
            nc.vector.tensor_add(
                out=bflat[:1, 0:512], in0=ps_b1[:1, 0:512], in1=qb_sb[:1, 0:512]
            )
            nc.vector.tensor_add(
                out=bflat[:1, 512:768], in0=ps_b2[:1, 0:256], in1=qb_sb[:1, 512:768]
            )
            bd = dramp.tile([1, 3 * C], F32)
            nc.scalar.dma_start(bd[:1, :], bflat[:1, :])
            bqk = small.tile([P, 3, CO], F32)
            nc.scalar.dma_start(
                bqk[:], bd[:][0].rearrange("(w co cp) -> cp w co", cp=P, w=3)
            )
            bqs = small.tile([P, CO], F32)  # rstd * bq, the Q-side additive
            nc.vector.tensor_scalar_mul(out=bqs[:], in0=bqk[:, 0, :], scalar1=rstd[:])
            bqs2 = bqs

            if _KNOB_STAGE < 4:
                return

            # ---------------- V^T = x^T (gamma-scaled Wv)  [j, c] ----------------
            NJB = HW // P  # 32 j-blocks
            VT_sb = big.tile([P, NJB, C], F32R)
            for jg in range(NJB // 4):
                ps = mmp.tile([P, CHW0], F32, tag="qk")
                for k4 in range(4):
                    jb = jg * 4 + k4
                    for co in range(CO):
                        nc.tensor.matmul(
                            ps[:, k4 * C : (k4 + 1) * C],
                            xps[jb // 8][:, co, (jb % 8) * P : (jb % 8 + 1) * P],
                            wqs[:, co, 512:768],
                            start=(co == 0),
                            stop=(co == CO - 1),
                        )
                if jg % 2 == 0:
                    nc.scalar.copy(
                        out=VT_sb[:, jg * 4 : (jg + 1) * 4, :], in_=ps[:]
                    )
                else:
                    nc.vector.tensor_copy(
                        VT_sb[:, jg * 4 : (jg + 1) * 4, :], ps[:]
                    )

            if _KNOB_STAGE < 5:
                return

            # ------------- Q~ = rstd^2 * (gamma-scaled Wq) xq + rstd*bq  [c, i] -----
            Q_sbs = [
                big.tile([P, CO, CHW0], F32R, tag=f"Qc{k}", name=f"Qc{k}")
                for k in range(NQ // CHW0)
            ]
            for ip in range(NQ // CHW0):
                for d2 in range(CO):
                    ps = mmp.tile([P, CHW0], F32, tag="qk")
                    for half in range(2):
                        for co in range(CO):
                            nc.tensor.matmul(
                                ps[:, half * 512 : (half + 1) * 512],
                                wqs[:, co, d2 * P : (d2 + 1) * P],
                                xqh[ip][:, co, half * 512 : (half + 1) * 512],
                                start=(co == 0),
                                stop=(co == CO - 1),
                            )
                    if d2 == 0:
                        nc.scalar.activation(
                            out=Q_sbs[ip][:, d2, :],
                            in_=ps[:],
                            func=Act.Identity,
                            bias=bqs2[:, d2 : d2 + 1],
                            scale=rs2[:],
                        )
                    else:
                        nc.vector.tensor_scalar(
                            out=Q_sbs[ip][:, d2, :],
                            in0=ps[:],
                            scalar1=rs2[:],
                            scalar2=bqs[:, d2 : d2 + 1],
                            op0=Alu.mult,
                            op1=Alu.add,
                        )

            # proj bias absorbs V bias: bpj = projb + Wp @ bv
            bqkv_r = small.tile([P, CO], F32R)
            nc.vector.tensor_copy(bqkv_r[:], bqk[:, 2, :])
            ps_pb = mmp.tile([P, CHW0], F32, tag="qk")
            for co in range(CO):
                nc.tensor.matmul(
                    ps_pb[:1, 0:C],
                    bqkv_r[:, co : co + 1],
                    wpj[:, co, :],
                    start=(co == 0),
                    stop=(co == CO - 1),
                )
            pb_flat = small.tile([1, C], F32)
            nc.vector.tensor_add(
                out=pb_flat[:1, :], in0=ps_pb[:1, 0:C], in1=pb_sb[:1, :]
            )
            bd2 = dramp.tile([1, C], F32)
            nc.scalar.dma_start(bd2[:1, :], pb_flat[:1, :])
            bpj = small.tile([P, CO], F32)
            nc.scalar.dma_start(bpj[:], bd2[:][0].rearrange("(co cp) -> cp co", cp=P))

            if _KNOB_STAGE < 6:
                return

            # ---------------- attention (streamed, transposed) ----------------
            # chunk widths [1024, 512, 512]: wide chunks amortize per-op
            # overheads; the narrow final chunks halve the exposed tail chain
            CHUNKS = [(0, 512), (512, 512), (1024, 512), (1536, 512)]
            ao_sbs = [
                big.tile([P, CO, cw], F32R, tag=f"aoc{k}", name=f"aoc{k}")
                for k, (_, cw) in enumerate(CHUNKS)
            ]

            def _emit_proj(pci):
                coff, cw = CHUNKS[pci]
                for i2 in range(cw // 512):
                    for d2 in range(CO):
                        s = slice(coff + i2 * 512, coff + (i2 + 1) * 512)
                        pps = mmp.tile([P, CHW0], F32, tag="qk", name="pps")
                        for co in range(CO):
                            nc.tensor.matmul(
                                pps[:, 0:512],
                                wpjs[:, co, d2 * P : (d2 + 1) * P],
                                ao_sbs[pci][:, co, i2 * 512 : (i2 + 1) * 512],
                                start=(co == 0),
                                stop=(co == CO - 1),
                            )
                        ot = outp.tile([P, 512], F32, tag="ot", name="ot")
                        nc.scalar.activation(
                            out=ot[:],
                            in_=pps[:, 0:512],
                            func=Act.Identity,
                            bias=bpj[:, d2 : d2 + 1],
                            scale=1.0,
                        )
                        g0 = coff + i2 * 512
                        yt = outp.tile([P, 512], F32, tag="yt", name="yt")
                        nc.vector.tensor_add(
                            out=yt[:],
                            in0=ot[:],
                            in1=xqh[g0 // 1024][
                                :, d2, g0 % 1024 : g0 % 1024 + 512
                            ].bitcast(F32),
                        )
                        nc.sync.dma_start(_co_view(y[:])[:, d2, s], yt[:])

            for ci, (coff, cw) in enumerate(CHUNKS):
                ni2 = cw // 512
                qi, qoff = coff // CHW0, coff % CHW0
                # denominator partials: even j-blocks on DVE, odd on GpSimd
                dacc_v = dap.tile([P, CHW0], F32R, tag="dacc_v")
                nc.vector.memset(dacc_v[:, 0:cw].bitcast(F32), 0.0)
                dacc_g = dap.tile([P, CHW0], F32, tag="dacc_g")
                nc.gpsimd.memset(dacc_g[:, 0:cw], 0.0)
                pvs = [
                    pvp.tile([P, 512], F32, tag=f"pv{k}", name=f"pv{k}")
                    for k in range(CO * ni2)
                ]
                exs = [None] * NJB
                # software-pipelined 2-deep: QK/exp for jb before PV for jb-2
                for jb in range(NJB + 5):
                    if jb < NJB:
                        qk = mmp.tile([P, CHW0], F32, tag="qk")
                        for i2 in range(ni2):
                            for co in range(CO):
                                nc.tensor.matmul(
                                    qk[:, i2 * 512 : (i2 + 1) * 512],
                                    K_sb[:, co, jb * P : (jb + 1) * P],
                                    Q_sbs[qi][
                                        :, co, qoff + i2 * 512 : qoff + (i2 + 1) * 512
                                    ],
                                    start=(co == 0),
                                    stop=(co == CO - 1),
                                )
                        ex = expp.tile([P, CHW0], F32R, tag="ex")
                        nc.scalar.activation(
                            out=ex[:, 0:cw], in_=qk[:, 0:cw], func=Act.Exp, scale=SCALE
                        )
                        if jb % 2 == 0:
                            nc.vector.tensor_add(
                                out=dacc_v[:, 0:cw],
                                in0=dacc_v[:, 0:cw].bitcast(F32),
                                in1=ex[:, 0:cw].bitcast(F32),
                            )
                        else:
                            nc.gpsimd.tensor_add(
                                out=dacc_g[:, 0:cw],
                                in0=dacc_g[:, 0:cw],
                                in1=ex[:, 0:cw].bitcast(F32),
                            )
                        exs[jb] = ex
                    if jb >= 5:
                        pj = jb - 5
                        for c2 in range(CO):
                            for i2 in range(ni2):
                                nc.tensor.matmul(
                                    pvs[c2 * ni2 + i2][:],
                                    VT_sb[:, pj, c2 * P : (c2 + 1) * P],
                                    exs[pj][:, i2 * 512 : (i2 + 1) * 512],
                                    start=(pj == 0),
                                    stop=(pj == NJB - 1),
                                )
                    if jb == 8 and ci > 0:
                        _emit_proj(ci - 1)

                dgr = dap.tile([P, CHW0], F32R, tag="dscr")
                nc.vector.tensor_copy(dgr[:, 0:cw], dacc_g[:, 0:cw])
                dps = mmp.tile([P, CHW0], F32, tag="qk")
                for i2 in range(ni2):
                    s = slice(i2 * 512, (i2 + 1) * 512)
                    nc.tensor.matmul(
                        dps[:, s], ones_r[:], dacc_v[:, s], start=True, stop=False
                    )
                    nc.tensor.matmul(
                        dps[:, s], ones_r[:], dgr[:, s], start=False, stop=True
                    )
                rec = dap.tile([P, CHW0], F32, tag="dscr")
                for i2 in range(ni2):
                    si = slice(i2 * 512, (i2 + 1) * 512)
                    nc.vector.reciprocal(out=rec[:, si], in_=dps[:, si])
                    for c2 in range(CO):
                        nc.vector.tensor_mul(
                            out=ao_sbs[ci][:, c2, si],
                            in0=pvs[c2 * ni2 + i2][:],
                            in1=rec[:, si],
                        )
                if ci == len(CHUNKS) - 1:
                    _emit_proj(ci)



def _get_prog():
    global _prog
    if _prog is None:
        _prog = _build()
    return _prog


def kernel(x, norm_gamma, norm_beta, qkv_w, qkv_b, proj_w, proj_b, **_ignored):
    x = np.ascontiguousarray(np.asarray(x, dtype=np.float32))
    gamma = np.ascontiguousarray(np.asarray(norm_gamma, dtype=np.float32))
    beta = np.ascontiguousarray(np.asarray(norm_beta, dtype=np.float32))
    qkv_b = np.ascontiguousarray(np.asarray(qkv_b, dtype=np.float32))
    proj_b = np.ascontiguousarray(np.asarray(proj_b, dtype=np.float32))
    wqkvT = np.ascontiguousarray(np.asarray(qkv_w, dtype=np.float32).T)
    wprojT = np.ascontiguousarray(np.asarray(proj_w, dtype=np.float32).T)

    nc = _get_prog()
    in_maps = []
    for c in range(NCORES):
        b, h = divmod(c, 2)
        xb = np.ascontiguousarray(x[b].reshape(C, HW))
        xqh = np.ascontiguousarray(xb[:, h * NQ : (h + 1) * NQ])
        in_maps.append(
            dict(
                xr=xb,
                xq=xqh,
                wqkvT=wqkvT,
                wprojT=wprojT,
                qkvb=qkv_b.reshape(1, 3 * C),
                projb=proj_b.reshape(1, C),
                gamma=gamma.reshape(1, C),
                beta=beta.reshape(1, C),
            )
        )
    res = run_bass_kernel_spmd(nc, in_maps, core_ids=list(range(NCORES)))
    out = np.empty((B, C, HW), np.float32)
    for c in range(NCORES):
        b, h = divmod(c, 2)
        out[b, :, h * NQ : (h + 1) * NQ] = res.results[c]["y"]
    return out.reshape(B, C, 64, 64)


# revision 63
# speedup vs baseline: 1.0970x; 1.0028x over previous
"""AttentionBlock (GroupNorm + single-head 1x1-conv attention + residual) on 8 trn2 cores.

Sharding: 8 cores = (batch b in 0..3) x (query-half h in 0..1). Each core computes
the full attention rows for its 2048 query positions of its sample.

Per-core pipeline (all matmuls in float32r: full PE rate for moving-dim >= 256,
near-fp32 precision in practice):
  1. x loads in pieces; GroupNorm(1 group) stats via bn_stats overlapped with the
     DMA; the cross-partition combine runs on GpSimd (partition_all_reduce) so
     the PE instruction stream is never blocked by statistics.
  2. qkv weights are scaled by gamma only (known immediately), so all QKV
     matmuls start as soon as x pieces land. The scalar rstd and the effective
     q-bias are folded into Q alone: Q~ = rstd^2*q_raw + rstd*bq. Then
     logits = Q~^T K_raw reproduces rstd^2*q.k + u[j]; the k-bias and the
     constant terms cancel inside softmax. K and V therefore get plain
     (stats-free) PSUM->SBUF rounding copies. The V-path rstd is folded into
     the proj weights, and the V bias exactly into the proj bias (softmax rows
     sum to 1): out = (rstd*Wp) @ ao_raw + (Wp @ bv + pb).
  3. Q[c, i] (own half), K[c, j] (full), V^T[j, c] (full, produced transposed
     directly by swapping matmul operands).
  4. Streamed attention over 32 j-blocks x 1024-wide i-chunks: logits^T[jb, i]
     = K_blk^T Q~ into a 2-bank PSUM tile, one fused scale+exp on ScalarE per
     block, denominator partials alternating VectorE/GpSimd (no port contention
     at 1x), PV accumulated over j-blocks in PSUM. Softmax max-subtraction is
     skipped: logits are bounded (|l| <= 16) so exp is safe in fp32 and the
     result is mathematically identical.
  5. Denominator replicated across partitions with a ones-matmul, reciprocal,
     applied to PV. proj + bias + residual per chunk, overlapping the next
     chunk's attention.
"""

import os

import numpy as np

import concourse.bass as bass
import concourse.bass_isa as bass_isa
import concourse.mybir as mybir
import concourse.tile as tile
from concourse import bacc
from concourse.bass_utils import run_bass_kernel_spmd

# perf-experiment knobs (only read at build time; defaults = production)
_KNOB_NCH = int(os.environ.get("K_NCH", "2"))  # attention i-chunks emitted
_KNOB_MMBUFS = int(os.environ.get("K_MMBUFS", "3"))
_KNOB_EXBUFS = int(os.environ.get("K_EXBUFS", "7"))
_KNOB_STAGE = int(os.environ.get("K_STAGE", "9"))

B, C, HW = 4, 256, 4096
CHW0 = 1024  # attention i-chunk width == qk psum tile width (2 banks)
P, CO = 128, 2
NQ = HW // 2  # queries per core
NCORES = 8
SCALE = float(C) ** -0.5
EPS = 1e-5
F32, F32R = mybir.dt.float32, mybir.dt.float32r
Act = mybir.ActivationFunctionType
Alu = mybir.AluOpType

_prog = None


def _co_view(ap):
    """[C, N] dram AP -> [128, 2, N] (partition cp, free (co, n)); c = co*128+cp."""
    return ap.rearrange("(co cp) n -> cp co n", cp=P)


def _build():
    nc = bacc.Bacc(None, target_bir_lowering=False)
    _build_body(nc)
    nc.compile()
    return nc


def _build_body(nc):
    xr = nc.dram_tensor("xr", [C, HW], F32, kind="ExternalInput")
    xq = nc.dram_tensor("xq", [C, NQ], F32, kind="ExternalInput")
    wqkvT = nc.dram_tensor("wqkvT", [C, 3 * C], F32, kind="ExternalInput")
    wprojT = nc.dram_tensor("wprojT", [C, C], F32, kind="ExternalInput")
    qkvb = nc.dram_tensor("qkvb", [1, 3 * C], F32, kind="ExternalInput")
    projb = nc.dram_tensor("projb", [1, C], F32, kind="ExternalInput")
    gamma = nc.dram_tensor("gamma", [1, C], F32, kind="ExternalInput")
    beta = nc.dram_tensor("beta", [1, C], F32, kind="ExternalInput")
    y = nc.dram_tensor("y", [C, NQ], F32, kind="ExternalOutput")

    with tile.TileContext(nc) as tc:
        with (
            tc.tile_pool(name="big", bufs=1) as big,
            tc.tile_pool(name="wts", bufs=1) as wts,
            tc.tile_pool(name="small", bufs=1) as small,
            tc.tile_pool(name="expp", bufs=_KNOB_EXBUFS) as expp,
            tc.tile_pool(name="dap", bufs=1) as dap,
            tc.tile_pool(name="outp", bufs=2) as outp,
            tc.tile_pool(name="mm", bufs=_KNOB_MMBUFS, space="PSUM") as mmp,
            tc.tile_pool(name="pv", bufs=1, space="PSUM") as pvp,
            tc.tile_pool(name="dram", bufs=1, space="DRAM") as dramp,
        ):
            # ---- x load as 4 separate piece tiles (whole-tile dep tracking:
            # separate tiles let bn/K/VT start as each piece lands) ----
            NXP = 4
            XPW = HW // NXP  # 1024
            xps = []
            stt = small.tile([P, 16, 6], F32)
            xqh = []
            for hh in range(2):
                t = big.tile([P, CO, NQ // 2], F32R, tag=f"xqh{hh}", name=f"xqh{hh}")
                xqh.append(t)
            for p_ in range(NXP):
                nsl = slice(p_ * XPW, (p_ + 1) * XPW)
                xp = big.tile([P, CO, XPW], F32R, tag=f"xp{p_}", name=f"xp{p_}")
                xps.append(xp)
                nc.sync.dma_start(xp[:], _co_view(xr[:])[:, :, nsl].bitcast(F32R))
                for co in range(CO):
                    for s2 in range(XPW // 512):
                        nc.vector.bn_stats(
                            out=stt[:, p_ * 4 + co * 2 + s2, :],
                            in_=xp[:, co, s2 * 512 : (s2 + 1) * 512].bitcast(F32),
                        )
                # interleave the xq half-loads between the stats pieces so the
                # Q matmuls are not starved behind the whole x transfer
                if p_ == 2 or p_ == 3:
                    hh = p_ - 2
                    nc.sync.dma_start(
                        xqh[hh][:],
                        _co_view(xq[:])[
                            :, :, hh * (NQ // 2) : (hh + 1) * (NQ // 2)
                        ].bitcast(F32R),
                    )

            # ---- weights (gamma-only scale: no stats dependency) ----
            # wq first on the scalar HWDGE ring: it gates the first K matmuls
            wq = wts.tile([P, CO, 3 * C], F32R)
            nc.sync.dma_start(wq[:], _co_view(wqkvT[:]).bitcast(F32R))
            gam = small.tile([P, CO], F32)
            nc.scalar.dma_start(gam[:], gamma[:][0].rearrange("(co cp) -> cp co", cp=P))
            bet = small.tile([P, CO], F32)
            nc.scalar.dma_start(bet[:], beta[:][0].rearrange("(co cp) -> cp co", cp=P))
            wqs = wts.tile([P, CO, 3 * C], F32R)
            for co in range(CO):
                nc.vector.tensor_scalar_mul(
                    out=wqs[:, co, :],
                    in0=wq[:, co, :].bitcast(F32),
                    scalar1=gam[:, co : co + 1],
                )
            wpj = wts.tile([P, CO, C], F32R)
            nc.scalar.dma_start(wpj[:], _co_view(wprojT[:]).bitcast(F32R))
            qb_sb = small.tile([1, 3 * C], F32)
            nc.scalar.dma_start(qb_sb[:1, :], qkvb[:])
            pb_sb = small.tile([1, C], F32)
            nc.scalar.dma_start(pb_sb[:1, :], projb[:])

            # ---- groupnorm stats (DVE + GpSimd only; PE never stalls) ----
            mv = small.tile([P, 2], F32)
            nc.vector.bn_aggr(out=mv[:], in_=stt[:])
            st2 = small.tile([P, 2], F32)
            nc.vector.tensor_mul(out=st2[:, 1:2], in0=mv[:, 0:1], in1=mv[:, 0:1])
            nc.vector.tensor_add(out=st2[:, 1:2], in0=st2[:, 1:2], in1=mv[:, 1:2])
            nc.vector.tensor_copy(st2[:, 0:1], mv[:, 0:1])
            st_red = small.tile([P, 2], F32)
            nc.gpsimd.partition_all_reduce(
                st_red[:], st2[:], channels=P, reduce_op=bass_isa.ReduceOp.add
            )
            meanv = small.tile([P, 1], F32)
            nc.scalar.mul(out=meanv[:], in_=st_red[:, 0:1], mul=1.0 / P)
            sqm = small.tile([P, 1], F32)
            nc.scalar.mul(out=sqm[:], in_=st_red[:, 1:2], mul=1.0 / P)
            msq = small.tile([P, 1], F32)
            nc.vector.tensor_mul(out=msq[:], in0=meanv[:], in1=meanv[:])
            varv = small.tile([P, 1], F32)
            nc.vector.tensor_sub(out=varv[:], in0=sqm[:], in1=msq[:])
            epsb = small.tile([P, 1], F32)
            nc.vector.memset(epsb[:], EPS)
            stdv = small.tile([P, 1], F32)
            nc.scalar.activation(
                out=stdv[:], in_=varv[:], func=Act.Sqrt, bias=epsb[:], scale=1.0
            )
            rstd = small.tile([P, 1], F32)
            nc.vector.reciprocal(out=rstd[:], in_=stdv[:])
            rs2 = small.tile([P, 1], F32)  # rstd^2 (for the Q fold)
            nc.vector.tensor_mul(out=rs2[:], in0=rstd[:], in1=rstd[:])
            # A = gamma * rstd ; Bterm = beta - mean * A
            A = small.tile([P, CO], F32)
            nc.vector.tensor_scalar_mul(out=A[:], in0=gam[:], scalar1=rstd[:])
            Bt = small.tile([P, CO], F32)
            nc.vector.tensor_scalar_mul(out=Bt[:], in0=A[:], scalar1=meanv[:])
            nc.vector.tensor_sub(out=Bt[:], in0=bet[:], in1=Bt[:])
            Br = small.tile([P, CO], F32R)
            nc.vector.tensor_copy(Br[:], Bt[:])
            # proj weights absorb the V-path rstd factor
            wpjs = wts.tile([P, CO, C], F32R)
            for co in range(CO):
                nc.vector.tensor_scalar_mul(
                    out=wpjs[:, co, :], in0=wpj[:, co, :].bitcast(F32), scalar1=rstd[:]
                )
            ones32 = small.tile([P, P], F32)
            nc.vector.memset(ones32[:], 1.0)
            ones_r = small.tile([P, P], F32R)
            nc.vector.tensor_copy(ones_r[:], ones32[:])

            if _KNOB_STAGE < 2:
                return

            # ---------------- K = gamma-scaled Wk x (raw)  [c, j] ----------------
            # plain rounding copies: k-bias and rstd live on the Q side
            K_sb = big.tile([P, CO, HW], F32R)
            for d2 in range(CO):
                for jp in range(HW // 1024):
                    ps = mmp.tile([P, CHW0], F32, tag="qk")
                    for half in range(2):
                        jt = jp * 2 + half
                        for co in range(CO):
                            nc.tensor.matmul(
                                ps[:, half * 512 : (half + 1) * 512],
                                wqs[:, co, 256 + d2 * P : 256 + (d2 + 1) * P],
                                xps[jt // 2][
                                    :, co, (jt % 2) * 512 : (jt % 2 + 1) * 512
                                ],
                                start=(co == 0),
                                stop=(co == CO - 1),
                            )
                    nc.scalar.copy(
                        out=K_sb[:, d2, jp * 1024 : (jp + 1) * 1024], in_=ps[:]
                    )

            if _KNOB_STAGE < 3:
                return

            # ---------------- effective qkv bias (per-partition via bounce) ----
            ps_b1 = mmp.tile([P, CHW0], F32, tag="qk")
            ps_b2 = mmp.tile([P, CHW0], F32, tag="qk")
            for co in range(CO):
                st, sp = (co == 0), (co == CO - 1)
                nc.tensor.matmul(
                    ps_b1[:1, 0:512],
                    Br[:, co : co + 1],
                    wq[:, co, 0:512],
                    start=st,
                    stop=sp,
                )
                nc.tensor.matmul(
                    ps_b2[:1, 0:256],
                    Br[:, co : co + 1],
                    wq[:, co, 512:768],
                    start=st,
                    stop=sp,
                )
            bflat = small.tile([1, 3 * C], F32)
            nc.vector.tensor_add(
                out=bflat[:1, 0:512], in0=ps_b1[:1, 0:512], in1=qb_sb[:1, 0:512]
            )
            nc.vector.tensor_add(
                out=bflat[:1, 512:768], in0=ps_b2[:1, 0:256], in1=qb_sb[:1, 512:768]
            )
            bd = dramp.tile([1, 3 * C], F32)
            nc.scalar.dma_start(bd[:1, :], bflat[:1, :])
            bqk = small.tile([P, 3, CO], F32)
            nc.scalar.dma_start(
                bqk[:], bd[:][0].rearrange("(w co cp) -> cp w co", cp=P, w=3)
            )
            bqs = small.tile([P, CO], F32)  # rstd * bq, the Q-side additive
            nc.vector.tensor_scalar_mul(out=bqs[:], in0=bqk[:, 0, :], scalar1=rstd[:])
            bqs2 = bqs

            if _KNOB_STAGE < 4:
                return

            # ---------------- V^T = x^T (gamma-scaled Wv)  [j, c] ----------------
            NJB = HW // P  # 32 j-blocks
            VT_sb = big.tile([P, NJB, C], F32R)
            for jg in range(NJB // 4):
                ps = mmp.tile([P, CHW0], F32, tag="qk")
                for k4 in range(4):
                    jb = jg * 4 + k4
                    for co in range(CO):
                        nc.tensor.matmul(
                            ps[:, k4 * C : (k4 + 1) * C],
                            xps[jb // 8][:, co, (jb % 8) * P : (jb % 8 + 1) * P],
                            wqs[:, co, 512:768],
                            start=(co == 0),
                            stop=(co == CO - 1),
                        )
                if jg % 2 == 0:
                    nc.scalar.copy(
                        out=VT_sb[:, jg * 4 : (jg + 1) * 4, :], in_=ps[:]
                    )
                else:
                    nc.vector.tensor_copy(
                        VT_sb[:, jg * 4 : (jg + 1) * 4, :], ps[:]
                    )

            if _KNOB_STAGE < 5:
                return

            # ------------- Q~ = rstd^2 * (gamma-scaled Wq) xq + rstd*bq  [c, i] -----
            Q_sbs = [
                big.tile([P, CO, CHW0], F32R, tag=f"Qc{k}", name=f"Qc{k}")
                for k in range(NQ // CHW0)
            ]
            for ip in range(NQ // CHW0):
                for d2 in range(CO):
                    ps = mmp.tile([P, CHW0], F32, tag="qk")
                    for half in range(2):
                        for co in range(CO):
                            nc.tensor.matmul(
                                ps[:, half * 512 : (half + 1) * 512],
                                wqs[:, co, d2 * P : (d2 + 1) * P],
                                xqh[ip][:, co, half * 512 : (half + 1) * 512],
                                start=(co == 0),
                                stop=(co == CO - 1),
                            )
                    if d2 == 0:
                        nc.scalar.activation(
                            out=Q_sbs[ip][:, d2, :],
                            in_=ps[:],
                            func=Act.Identity,
                            bias=bqs2[:, d2 : d2 + 1],
                            scale=rs2[:],
                        )
                    else:
                        nc.vector.tensor_scalar(
                            out=Q_sbs[ip][:, d2, :],
                            in0=ps[:],
                            scalar1=rs2[:],
                            scalar2=bqs[:, d2 : d2 + 1],
                            op0=Alu.mult,
                            op1=Alu.add,
                        )

            # proj bias absorbs V bias: bpj = projb + Wp @ bv
            bqkv_r = small.tile([P, CO], F32R)
            nc.vector.tensor_copy(bqkv_r[:], bqk[:, 2, :])
            ps_pb = mmp.tile([P, CHW0], F32, tag="qk")
            for co in range(CO):
                nc.tensor.matmul(
                    ps_pb[:1, 0:C],
                    bqkv_r[:, co : co + 1],
                    wpj[:, co, :],
                    start=(co == 0),
                    stop=(co == CO - 1),
                )
            pb_flat = small.tile([1, C], F32)
            nc.vector.tensor_add(
                out=pb_flat[:1, :], in0=ps_pb[:1, 0:C], in1=pb_sb[:1, :]
            )
            bd2 = dramp.tile([1, C], F32)
            nc.scalar.dma_start(bd2[:1, :], pb_flat[:1, :])
            bpj = small.tile([P, CO], F32)
            nc.scalar.dma_start(bpj[:], bd2[:][0].rearrange("(co cp) -> cp co", cp=P))

            if _KNOB_STAGE < 6:
                return

            # ---------------- attention (streamed, transposed) ----------------
            # chunk widths [1024, 512, 512]: wide chunks amortize per-op
            # overheads; the narrow final chunks halve the exposed tail chain
            CHUNKS = [(0, 512), (512, 512), (1024, 512), (1536, 512)]
            ao_sbs = [
                big.tile([P, CO, cw], F32R, tag=f"aoc{k}", name=f"aoc{k}")
                for k, (_, cw) in enumerate(CHUNKS)
            ]

            def _emit_proj(pci):
                coff, cw = CHUNKS[pci]
                for i2 in range(cw // 512):
                    for d2 in range(CO):
                        s = slice(coff + i2 * 512, coff + (i2 + 1) * 512)
                        pps = mmp.tile([P, CHW0], F32, tag="qk", name="pps")
                        for co in range(CO):
                            nc.tensor.matmul(
                                pps[:, 0:512],
                                wpjs[:, co, d2 * P : (d2 + 1) * P],
                                ao_sbs[pci][:, co, i2 * 512 : (i2 + 1) * 512],
                                start=(co == 0),
                                stop=(co == CO - 1),
                            )
                        ot = outp.tile([P, 512], F32, tag="ot", name="ot")
                        nc.scalar.activation(
                            out=ot[:],
                            in_=pps[:, 0:512],
                            func=Act.Identity,
                            bias=bpj[:, d2 : d2 + 1],
                            scale=1.0,
                        )
                        g0 = coff + i2 * 512
                        yt = outp.tile([P, 512], F32, tag="yt", name="yt")
                        nc.vector.tensor_add(
                            out=yt[:],
                            in0=ot[:],
                            in1=xqh[g0 // 1024][
                                :, d2, g0 % 1024 : g0 % 1024 + 512
                            ].bitcast(F32),
                        )
                        nc.sync.dma_start(_co_view(y[:])[:, d2, s], yt[:])

            for ci, (coff, cw) in enumerate(CHUNKS):
                ni2 = cw // 512
                qi, qoff = coff // CHW0, coff % CHW0
                # denominator partials: even j-blocks on DVE, odd on GpSimd
                dacc_v = dap.tile([P, CHW0], F32R, tag="dacc_v")
                nc.vector.memset(dacc_v[:, 0:cw].bitcast(F32), 0.0)
                dacc_g = dap.tile([P, CHW0], F32, tag="dacc_g")
                nc.gpsimd.memset(dacc_g[:, 0:cw], 0.0)
                pvs = [
                    pvp.tile([P, 512], F32, tag=f"pv{k}", name=f"pv{k}")
                    for k in range(CO * ni2)
                ]
                exs = [None] * NJB
                # software-pipelined 2-deep: QK/exp for jb before PV for jb-2
                for jb in range(NJB + 5):
                    if jb < NJB:
                        qk = mmp.tile([P, CHW0], F32, tag="qk")
                        for i2 in range(ni2):
                            for co in range(CO):
                                nc.tensor.matmul(
                                    qk[:, i2 * 512 : (i2 + 1) * 512],
                                    K_sb[:, co, jb * P : (jb + 1) * P],
                                    Q_sbs[qi][
                                        :, co, qoff + i2 * 512 : qoff + (i2 + 1) * 512
                                    ],
                                    start=(co == 0),
                                    stop=(co == CO - 1),
                                )
                        ex = expp.tile([P, CHW0], F32R, tag="ex")
                        nc.scalar.activation(
                            out=ex[:, 0:cw], in_=qk[:, 0:cw], func=Act.Exp, scale=SCALE
                        )
                        if jb % 2 == 0:
                            nc.vector.tensor_add(
                                out=dacc_v[:, 0:cw],
                                in0=dacc_v[:, 0:cw].bitcast(F32),
                                in1=ex[:, 0:cw].bitcast(F32),
                            )
                        else:
                            nc.gpsimd.tensor_add(
                                out=dacc_g[:, 0:cw],
                                in0=dacc_g[:, 0:cw],
                                in1=ex[:, 0:cw].bitcast(F32),
                            )
                        exs[jb] = ex
                    if jb >= 5:
                        pj = jb - 5
                        for c2 in range(CO):
                            for i2 in range(ni2):
                                nc.tensor.matmul(
                                    pvs[c2 * ni2 + i2][:],
                                    VT_sb[:, pj, c2 * P : (c2 + 1) * P],
                                    exs[pj][:, i2 * 512 : (i2 + 1) * 512],
                                    start=(pj == 0),
                                    stop=(pj == NJB - 1),
                                )
                    if jb == 8 and ci > 0:
                        _emit_proj(ci - 1)

                dgr = dap.tile([P, CHW0], F32R, tag="dscr")
                nc.vector.tensor_copy(dgr[:, 0:cw], dacc_g[:, 0:cw])
                dps = mmp.tile([P, CHW0], F32, tag="qk")
                for i2 in range(ni2):
                    s = slice(i2 * 512, (i2 + 1) * 512)
                    nc.tensor.matmul(
                        dps[:, s], ones_r[:], dacc_v[:, s], start=True, stop=False
                    )
                    nc.tensor.matmul(
                        dps[:, s], ones_r[:], dgr[:, s], start=False, stop=True
                    )
                rec = dap.tile([P, CHW0], F32, tag="dscr")
                for i2 in range(ni2):
                    si = slice(i2 * 512, (i2 + 1) * 512)
                    nc.vector.reciprocal(out=rec[:, si], in_=dps[:, si])
                    for c2 in range(CO):
                        nc.vector.tensor_mul(
                            out=ao_sbs[ci][:, c2, si],
                            in0=pvs[c2 * ni2 + i2][:],
                            in1=rec[:, si],
                        )
                if ci == len(CHUNKS) - 1:
                    _emit_proj(ci)



def _get_prog():
    global _prog
    if _prog is None:
        _prog = _build()
    return _prog


def kernel(x, norm_gamma, norm_beta, qkv_w, qkv_b, proj_w, proj_b, **_ignored):
    x = np.ascontiguousarray(np.asarray(x, dtype=np.float32))
    gamma = np.ascontiguousarray(np.asarray(norm_gamma, dtype=np.float32))
    beta = np.ascontiguousarray(np.asarray(norm_beta, dtype=np.float32))
    qkv_b = np.ascontiguousarray(np.asarray(qkv_b, dtype=np.float32))
    proj_b = np.ascontiguousarray(np.asarray(proj_b, dtype=np.float32))
    wqkvT = np.ascontiguousarray(np.asarray(qkv_w, dtype=np.float32).T)
    wprojT = np.ascontiguousarray(np.asarray(proj_w, dtype=np.float32).T)

    nc = _get_prog()
    in_maps = []
    for c in range(NCORES):
        b, h = divmod(c, 2)
        xb = np.ascontiguousarray(x[b].reshape(C, HW))
        xqh = np.ascontiguousarray(xb[:, h * NQ : (h + 1) * NQ])
        in_maps.append(
            dict(
                xr=xb,
                xq=xqh,
                wqkvT=wqkvT,
                wprojT=wprojT,
                qkvb=qkv_b.reshape(1, 3 * C),
                projb=proj_b.reshape(1, C),
                gamma=gamma.reshape(1, C),
                beta=beta.reshape(1, C),
            )
        )
    res = run_bass_kernel_spmd(nc, in_maps, core_ids=list(range(NCORES)))
    out = np.empty((B, C, HW), np.float32)
    for c in range(NCORES):
        b, h = divmod(c, 2)
        out[b, :, h * NQ : (h + 1) * NQ] = res.results[c]["y"]
    return out.reshape(B, C, 64, 64)
